# revision 6
# baseline (speedup 1.0000x reference)
"""Trainium2 Bass kernel for BasicQuadRGBModel (quad-Bayer demosaic CNN).

v2 layout (engine APs need partition base in {0,32,64,96}; DMA is exempt):
  - im2col buffers R [120p, 10 rows, 64 win]: main block xa=1..8 at partitions
    (xa-1)*12+ci = [0:96); xa=0 strip at [96:108); xa=9 strip at [108:120).
    PSUM eviction is then a base0->base0 relu copy; strips are SBUF->SBUF DMAs.
  - grb/d buffers [20p]: main (xa-1)*2+c at [0:16); strips [16:18),[18:20).
  - layer-0 im2col r0 [128p] host-built: ky0 block [0:40), ky1-other [40:60),
    ky1-rb [64:84) (aligned: feeds d_buf copies), ky2 block [84:124).
  - conv = 3 accumulating matmuls/layer (K=120, M=96, N=512 = 8 rows x 64 win);
    softmax/green/chroma folded into small matmuls; float32r for full PE rate.
  - host does layer-0 im2col and the final 2x2 pixel-shuffle.
"""

import sys

sys.path.insert(0, "/opt/trn_rl_repo")

import numpy as np

import concourse.bass as bass
import concourse.mybir as mybir
import concourse.tile as tile
from concourse import bacc
from concourse.bass_utils import run_bass_kernel_spmd

N_CORES = 8
B_PC = 2
H = W = 512
NW = 64
NSLAB = 64
CH = 12
F32 = mybir.dt.float32
F32R = mybir.dt.float32r
# dtype for every buffer that feeds a matmul (BIR requires producer dtype
# to match the matmul's f32r operands end-to-end; f32r = fp32 truncated to
# ~fp22 on the PE read path, 4x matmul throughput at N>=256)
MMDT = F32R


def _rbloc(xa, c):
    if xa == 0:
        return 16 + c
    if xa == 9:
        return 18 + c
    return (xa - 1) * 2 + c


def _rloc(xa, ci):
    if xa == 0:
        return 96 + ci
    if xa == 9:
        return 108 + ci
    return (xa - 1) * 12 + ci


def _r0loc(ky, ci, xa):
    if ky == 0:
        if ci == 0:
            return xa
        if ci == 3:
            return 10 + xa
        return 20 + _rbloc(xa, ci - 1)
    if ky == 1:
        if ci == 0:
            return 40 + xa
        if ci == 3:
            return 50 + xa
        return 64 + _rbloc(xa, ci - 1)
    if ci == 0:
        return 84 + xa
    if ci == 3:
        return 94 + xa
    return 104 + _rbloc(xa, ci - 1)


def build_r0(mosaic):
    B = mosaic.shape[0]
    mp = np.zeros((B, 4, H + 2, W + 2), np.float32)
    mp[:, :, 1 : H + 1, 1 : W + 1] = mosaic
    r0 = np.zeros((B, 128, H, NW), np.float32)
    for ky in range(3):
        for ci in range(4):
            for xa in range(10):
                r0[:, _r0loc(ky, ci, xa)] = mp[:, ci, ky : ky + H, xa : xa + 8 * NW : 8]
    return r0


def build_w_l0(wt):
    W_ = np.zeros((128, 96), np.float32)
    for ky in range(3):
        for ci in range(4):
            for xa in range(10):
                for xo in range(8):
                    kx = xa - xo
                    if 0 <= kx <= 2:
                        for co in range(CH):
                            W_[_r0loc(ky, ci, xa), xo * 12 + co] = wt[co, ci, ky, kx]
    return W_


def build_w_int(wt):
    W_ = np.zeros((3, 120, 96), np.float32)
    for ky in range(3):
        for xa in range(10):
            for xo in range(8):
                kx = xa - xo
                if 0 <= kx <= 2:
                    k = _rloc(xa, 0)
                    W_[ky, k : k + 12, xo * 12 : xo * 12 + 12] = wt[:, :, ky, kx].T
    return W_


def build_w_sums():
    wse = np.zeros((96, 8), np.float32)
    wsep = np.zeros((96, 16), np.float32)
    wbc = np.zeros((8, 16), np.float32)
    for xo in range(8):
        for co in range(CH):
            wse[xo * 12 + co, xo] = 1.0
            wsep[xo * 12 + co, xo * 2 + (co >= 6)] = 1.0
        wbc[xo, xo * 2 : xo * 2 + 2] = 1.0
    return wse, wsep, wbc


def build_w_chroma(cw0):
    wchk = np.zeros((3, 20, 48), np.float32)
    for ky in range(3):
        for xa in range(10):
            for xo in range(8):
                kx = xa - xo
                if 0 <= kx <= 2:
                    for co in range(6):
                        for d in range(2):
                            wchk[ky, _rbloc(xa, d), xo * 6 + co] = cw0[co, d, ky, kx]
    # green_add = [m0, g1, m3, m0, g0, m3]; g0 = m1 - d0, g1 = m2 - d1
    for xo in range(8):
        wchk[1, _rbloc(xo + 1, 1), xo * 6 + 1] += -1.0
        wchk[1, _rbloc(xo + 1, 0), xo * 6 + 4] += -1.0
    wchm = np.zeros((128, 48), np.float32)
    for xo in range(8):
        xa = xo + 1
        wchm[_r0loc(1, 0, xa), xo * 6 + 0] = 1.0
        wchm[_r0loc(1, 0, xa), xo * 6 + 3] = 1.0
        wchm[_r0loc(1, 3, xa), xo * 6 + 2] = 1.0
        wchm[_r0loc(1, 3, xa), xo * 6 + 5] = 1.0
        wchm[_r0loc(1, 2, xa), xo * 6 + 1] = 1.0
        wchm[_r0loc(1, 1, xa), xo * 6 + 4] = 1.0
    return wchk, wchm


def assemble_output(mosaic, cp_dev, g_dev):
    B = mosaic.shape[0]
    cp = cp_dev.reshape(B, 8, 6, H, NW).transpose(0, 2, 3, 4, 1).reshape(B, 6, H, W)
    g = g_dev.reshape(B, 8, 2, H, NW).transpose(0, 2, 3, 4, 1).reshape(B, 2, H, W)
    m = mosaic
    out = np.empty((B, 3, 2 * H, 2 * W), np.float32)
    out[:, 0, 0::2, 0::2] = cp[:, 0]
    out[:, 0, 0::2, 1::2] = m[:, 1]
    out[:, 0, 1::2, 0::2] = cp[:, 1]
    out[:, 0, 1::2, 1::2] = cp[:, 2]
    out[:, 1, 0::2, 0::2] = m[:, 0]
    out[:, 1, 0::2, 1::2] = g[:, 0]
    out[:, 1, 1::2, 0::2] = g[:, 1]
    out[:, 1, 1::2, 1::2] = m[:, 3]
    out[:, 2, 0::2, 0::2] = cp[:, 3]
    out[:, 2, 0::2, 1::2] = cp[:, 4]
    out[:, 2, 1::2, 0::2] = m[:, 2]
    out[:, 2, 1::2, 1::2] = cp[:, 5]
    return out


def _mm_dt(ap):
    return ap


# column offsets inside the packed [128, 1576] stationary tensor
_WOFF = {"wf0": 0, "ww0": 96, "wf1": 192, "wf2": 480, "ww1": 768, "ww2": 1056,
         "wse": 1344, "wsep": 1352, "wbc": 1368, "wchk": 1384, "wchm": 1528}
_WCOLS = 1576


def pack_stationaries(st):
    wp = np.zeros((128, _WCOLS), np.float32)
    wp[:, 0:96] = st["wf0"]
    wp[:, 96:192] = st["ww0"]
    for nm in ("wf1", "wf2", "ww1", "ww2"):
        o = _WOFF[nm]
        for ky in range(3):
            wp[0:120, o + 96 * ky : o + 96 * (ky + 1)] = st[nm][ky]
    wp[0:96, 1344:1352] = st["wse"]
    wp[0:96, 1352:1368] = st["wsep"]
    wp[0:8, 1368:1384] = st["wbc"]
    for ky in range(3):
        wp[0:20, 1384 + 48 * ky : 1384 + 48 * (ky + 1)] = st["wchk"][ky]
    wp[:, 1528:1576] = st["wchm"]
    return wp


_W_SHAPES = [
    ("wf0", [128, 96]),
    ("ww0", [128, 96]),
    ("wf1", [120, 3, 96]),
    ("wf2", [120, 3, 96]),
    ("ww1", [120, 3, 96]),
    ("ww2", [120, 3, 96]),
    ("wse", [96, 8]),
    ("wsep", [96, 16]),
    ("wbc", [8, 16]),
    ("wchk", [20, 3, 48]),
    ("wchm", [128, 48]),
]


def build_program():
    from contextlib import ExitStack

    nc = bacc.Bacc(
        "TRN2", target_bir_lowering=False, debug=False, num_devices=N_CORES
    )
    r0 = nc.declare_dram_parameter("r0", [B_PC, 128, H, NW], MMDT, isOutput=False)
    wpack = nc.declare_dram_parameter("wpack", [128, _WCOLS], MMDT, isOutput=False)
    out_cp = nc.declare_dram_parameter("out_cp", [B_PC, 48, H, NW], F32, isOutput=True)
    out_g = nc.declare_dram_parameter("out_g", [B_PC, 16, H, NW], MMDT, isOutput=True)

    Relu = mybir.ActivationFunctionType.Relu
    Exp = mybir.ActivationFunctionType.Exp
    Copy = mybir.ActivationFunctionType.Copy
    NSTEPS = B_PC * NSLAB

    with tile.TileContext(nc) as tc, ExitStack() as ctx:
        const = ctx.enter_context(tc.tile_pool(name="const", bufs=1))
        r0pool = ctx.enter_context(tc.tile_pool(name="r0pool", bufs=6))
        p_rf1 = ctx.enter_context(tc.tile_pool(name="rf1", bufs=4))
        p_rw1 = ctx.enter_context(tc.tile_pool(name="rw1", bufs=4))
        p_rf2 = ctx.enter_context(tc.tile_pool(name="rf2", bufs=4))
        p_rw2 = ctx.enter_context(tc.tile_pool(name="rw2", bufs=4))
        p_grb = ctx.enter_context(tc.tile_pool(name="grb", bufs=4))
        p_d = ctx.enter_context(tc.tile_pool(name="dbuf", bufs=2))
        p_act = ctx.enter_context(tc.tile_pool(name="acts", bufs=3))
        p_stg = ctx.enter_context(tc.tile_pool(name="stg", bufs=3))
        ps_mm = ctx.enter_context(tc.tile_pool(name="psmm", bufs=4, space="PSUM"))
        ps_sm = ctx.enter_context(tc.tile_pool(name="pssm", bufs=2, space="PSUM"))
        ps_cp = ctx.enter_context(tc.tile_pool(name="pscp", bufs=2, space="PSUM"))

        WC = const.tile([128, _WCOLS], MMDT, tag="wpack_sb", name="wpack_sb")
        nc.sync.dma_start(out=WC[:], in_=wpack[:])
        sb = {
            "wf0": WC[:, 0:96],
            "ww0": WC[:, 96:192],
            "wse": WC[0:96, 1344:1352],
            "wsep": WC[0:96, 1352:1368],
            "wbc": WC[0:8, 1368:1384],
            "wchm": WC[:, 1528:1576],
        }

        def wky(nm, ky):
            o = _WOFF[nm]
            if nm == "wchk":
                return WC[0:20, o + 48 * ky : o + 48 * (ky + 1)]
            return WC[0:120, o + 96 * ky : o + 96 * (ky + 1)]

        r0s, rf1, rw1, rf2, rw2, grb = {}, {}, {}, {}, {}, {}

        def get_rbuf(pool, dct, s):
            if s in dct or not (0 <= s < NSTEPS):
                return dct.get(s)
            t = pool.tile([120, 10, NW], MMDT)
            dct[s] = t
            sl = s % NSLAB
            if sl == 0:
                nc.vector.memset(t[:, 0:1, :].bitcast(F32), 0.0)
            if sl == NSLAB - 1:
                nc.vector.memset(t[:, 9:10, :].bitcast(F32), 0.0)
            nc.vector.memset(t[96:120, :, 0:1].bitcast(F32), 0.0)
            nc.vector.memset(t[96:120, :, 63:64].bitcast(F32), 0.0)
            return t

        def get_grb(s):
            if s in grb or not (0 <= s < NSTEPS):
                return grb.get(s)
            t = p_grb.tile([20, 10, NW], MMDT, name="g")
            grb[s] = t
            sl = s % NSLAB
            if sl == 0:
                nc.vector.memset(t[:, 0:1, :].bitcast(F32), 0.0)
            if sl == NSLAB - 1:
                nc.vector.memset(t[:, 9:10, :].bitcast(F32), 0.0)
            nc.vector.memset(t[:, :, 0:1].bitcast(F32), 0.0)
            nc.vector.memset(t[:, :, 63:64].bitcast(F32), 0.0)
            return t

        def conv_int(nm, rbuf):
            ps = ps_mm.tile([96, 8, NW], F32, tag="mm96", name="psc")
            for ky in range(3):
                nc.tensor.matmul(
                    ps[:],
                    _mm_dt(wky(nm, ky)),
                    _mm_dt(rbuf[:, ky : ky + 8, :]),
                    start=(ky == 0),
                    stop=(ky == 2),
                )
            return ps

        def evict(ps, dct, s):
            sl = s % NSLAB
            nc.scalar.activation(out=dct[s][0:96, 1:9, :], in_=ps[:], func=Relu)
            if sl < NSLAB - 1:
                nc.scalar.activation(
                    out=dct[s + 1][0:96, 0:1, :], in_=ps[:, 7:8, :], func=Relu
                )
            if sl > 0:
                nc.scalar.activation(
                    out=dct[s - 1][0:96, 9:10, :], in_=ps[:, 0:1, :], func=Relu
                )

        def strips(t):
            nc.sync.dma_start(out=t[96:108, :, 1:NW], in_=t[84:96, :, 0 : NW - 1])
            nc.sync.dma_start(out=t[108:120, :, 0 : NW - 1], in_=t[0:12, :, 1:NW])

        for T in range(NSTEPS + 3):
            s0 = T
            if 0 <= s0 < NSTEPS:
                img, sl = divmod(s0, NSLAB)
                y0 = sl * 8
                rt = r0pool.tile([128, 8, NW], MMDT, name="rt")
                r0s[s0] = rt
                nc.sync.dma_start(out=rt[:], in_=r0[img, :, y0 : y0 + 8, :])
                get_rbuf(p_rf1, rf1, s0)
                get_rbuf(p_rf1, rf1, s0 + 1)
                get_rbuf(p_rw1, rw1, s0)
                get_rbuf(p_rw1, rw1, s0 + 1)
                psf = ps_mm.tile([96, 8, NW], F32, tag="mm96", name="psf0")
                nc.tensor.matmul(
                    psf[:], _mm_dt(sb["wf0"]), _mm_dt(rt[:]), start=True, stop=True
                )
                evict(psf, rf1, s0)
                psw = ps_mm.tile([96, 8, NW], F32, tag="mm96", name="psw0")
                nc.tensor.matmul(
                    psw[:], _mm_dt(sb["ww0"]), _mm_dt(rt[:]), start=True, stop=True
                )
                evict(psw, rw1, s0)

            s1 = T - 1
            if 0 <= s1 < NSTEPS:
                strips(rf1[s1])
                strips(rw1[s1])
                get_rbuf(p_rf2, rf2, s1)
                get_rbuf(p_rf2, rf2, s1 + 1)
                get_rbuf(p_rw2, rw2, s1)
                get_rbuf(p_rw2, rw2, s1 + 1)
                evict(conv_int("wf1", rf1[s1]), rf2, s1)
                evict(conv_int("ww1", rw1[s1]), rw2, s1)

            s2 = T - 2
            if 0 <= s2 < NSTEPS:
                strips(rf2[s2])
                strips(rw2[s2])
                psf = conv_int("wf2", rf2[s2])
                psw = conv_int("ww2", rw2[s2])
                P = p_act.tile([96, 8, NW], F32, tag="P", name="P")
                nc.scalar.activation(out=P[:], in_=psf[:], func=Relu)
                Et = p_act.tile([96, 8, NW], F32, tag="Et", name="Et")
                nc.scalar.activation(out=Et[:], in_=psw[:], func=Relu)
                E = p_act.tile([96, 8, NW], MMDT, tag="E", name="E")
                nc.scalar.activation(out=E[:], in_=Et[:], func=Exp)
                EP = p_act.tile([96, 8, NW], MMDT, tag="EP", name="EP")
                nc.vector.tensor_mul(EP[:], E[:], P[:])
                pse = ps_sm.tile([8, 8, NW], F32, tag="sm", name="pse")
                nc.tensor.matmul(
                    pse[:], _mm_dt(sb["wse"]), _mm_dt(E[:]), start=True, stop=True
                )
                psep = ps_sm.tile([16, 8, NW], F32, tag="sm", name="psep")
                nc.tensor.matmul(
                    psep[:], _mm_dt(sb["wsep"]), _mm_dt(EP[:]), start=True, stop=True
                )
                rcp = p_act.tile([8, 8, NW], MMDT, tag="rcp", name="rcp")
                with nc.allow_low_precision(reason="f32r ~ fp22, tol 2e-2"):
                    nc.vector.reciprocal(out=rcp[:], in_=pse[:])
                psbc = ps_sm.tile([16, 8, NW], F32, tag="sm", name="psbc")
                nc.tensor.matmul(
                    psbc[:], _mm_dt(sb["wbc"]), _mm_dt(rcp[:]), start=True, stop=True
                )
                bcs = p_act.tile([16, 8, NW], F32, tag="bcs", name="bcs")
                nc.scalar.activation(out=bcs[:], in_=psbc[:], func=Copy)
                get_grb(s2)
                get_grb(s2 + 1)
                g = grb[s2]
                nc.vector.tensor_mul(g[0:16, 1:9, :], psep[:], bcs[:])
                sl = s2 % NSLAB
                if sl < NSLAB - 1:
                    nc.vector.tensor_copy(
                        out=grb[s2 + 1][0:16, 0:1, :], in_=g[0:16, 8:9, :]
                    )
                if sl > 0:
                    nc.vector.tensor_copy(
                        out=grb[s2 - 1][0:16, 9:10, :], in_=g[0:16, 1:2, :]
                    )

            s3 = T - 3
            if 0 <= s3 < NSTEPS:
                img, sl = divmod(s3, NSLAB)
                y0 = sl * 8
                g = grb[s3]
                nc.sync.dma_start(out=g[16:18, :, 1:NW], in_=g[14:16, :, 0 : NW - 1])
                nc.sync.dma_start(out=g[18:20, :, 0 : NW - 1], in_=g[0:2, :, 1:NW])
                rt = r0s[s3]
                d = p_d.tile([20, 10, NW], MMDT, name="d")
                nc.vector.tensor_copy(out=d[:, 1:9, :], in_=rt[64:84, :, :])
                if sl > 0:
                    nc.vector.tensor_copy(
                        out=d[:, 0:1, :], in_=r0s[s3 - 1][64:84, 7:8, :]
                    )
                else:
                    nc.vector.memset(d[:, 0:1, :].bitcast(F32), 0.0)
                if sl < NSLAB - 1:
                    nc.vector.tensor_copy(
                        out=d[:, 9:10, :], in_=r0s[s3 + 1][64:84, 0:1, :]
                    )
                else:
                    nc.vector.memset(d[:, 9:10, :].bitcast(F32), 0.0)
                nc.vector.tensor_sub(d[:], d[:], g[:])
                pc = ps_cp.tile([48, 8, NW], F32, tag="cp", name="pc")
                for ky in range(3):
                    nc.tensor.matmul(
                        pc[:],
                        _mm_dt(wky("wchk", ky)),
                        _mm_dt(d[:, ky : ky + 8, :]),
                        start=(ky == 0),
                        stop=False,
                    )
                nc.tensor.matmul(
                    pc[:], _mm_dt(sb["wchm"]), _mm_dt(rt[:]), start=False, stop=True
                )
                stg = p_stg.tile([48, 8, NW], F32, name="stg")
                nc.scalar.activation(out=stg[:], in_=pc[:], func=Copy)
                nc.sync.dma_start(out=out_cp[img, :, y0 : y0 + 8, :], in_=stg[:])
                nc.sync.dma_start(out=out_g[img, :, y0 : y0 + 8, :], in_=g[0:16, 1:9, :])
                for dct in (r0s, rf1, rw1, rf2, rw2, grb):
                    dct.pop(s3 - 2, None)

    nc.compile()
    return nc


_CACHE = {}


def kernel(mosaic, fw0, fw1, fw2, ww0, ww1, ww2, cw0, _trace=False):
    mosaic = np.asarray(mosaic, np.float32)
    r0_all = build_r0(mosaic)

    stat = {
        "wf0": build_w_l0(np.asarray(fw0, np.float32)),
        "ww0": build_w_l0(np.asarray(ww0, np.float32)),
        "wf1": build_w_int(np.asarray(fw1, np.float32)),
        "wf2": build_w_int(np.asarray(fw2, np.float32)),
        "ww1": build_w_int(np.asarray(ww1, np.float32)),
        "ww2": build_w_int(np.asarray(ww2, np.float32)),
    }
    stat["wse"], stat["wsep"], stat["wbc"] = build_w_sums()
    stat["wchk"], stat["wchm"] = build_w_chroma(np.asarray(cw0, np.float32))
    wpack = pack_stationaries(stat)

    if "nc" not in _CACHE:
        _CACHE["nc"] = build_program()
    nc = _CACHE["nc"]

    in_maps = []
    for c in range(N_CORES):
        in_maps.append(
            {"r0": np.ascontiguousarray(r0_all[c * B_PC : (c + 1) * B_PC]),
             "wpack": wpack}
        )

    res = run_bass_kernel_spmd(nc, in_maps, list(range(N_CORES)), trace=_trace)
    outs = []
    for c in range(N_CORES):
        outs.append(
            assemble_output(
                mosaic[c * B_PC : (c + 1) * B_PC],
                res.results[c]["out_cp"],
                res.results[c]["out_g"],
            )
        )
    full = np.concatenate(outs, axis=0)
    if _trace:
        return full, res
    return full



# revision 7
# speedup vs baseline: 1.3292x; 1.3292x over previous
"""Trainium2 Bass kernel for BasicQuadRGBModel (quad-Bayer demosaic CNN).

v4: bf16 end-to-end (1 cyc/row matmuls, halved DMA, DVE 2x/4x modes).
  - im2col buffers R [120p, 10 rows, 64 win]: main block xa=1..8 at partitions
    (xa-1)*12+ci = [0:96); xa=0 strip at [96:108); xa=9 strip at [108:120).
    f-branch PSUM evictions on ACT (relu), w-branch on DVE (tensor_relu);
    x-halo strips are SBUF->SBUF DMAs.
  - grb buffer [20p]: main (xa-1)*2+c at [0:16); strips [16:18),[18:20).
  - layer-0 im2col r0 [128p] host-built (bf16): ky0 block [0:40), ky1-other
    [40:60), ky1-rb [64:84), ky2 block [84:124).
  - conv = 3 accumulating matmuls/layer (K=120, M=96, N=512 = 8 rows x 64 win).
  - chroma conv(rb-g) folded host-side: rb-taps scattered into the wchm matmul
    on r0 (rb rows are already replicated there per ky); g-taps via 3 negated
    banded matmuls on grb. No on-chip d=rb-g buffer at all.
  - softmax: E=max(exp(psw),1) (ACT exp + DVE max); EP=max(psf*E,0) (DVE);
    1/sum via ACT exp(-ln(sum)); broadcast via tiny K=8 matmul.
  - host does layer-0 im2col and the final 2x2 pixel-shuffle.
"""

import sys

sys.path.insert(0, "/opt/trn_rl_repo")

import ml_dtypes
import numpy as np

import concourse.bass as bass
import concourse.mybir as mybir
import concourse.tile as tile
from concourse import bacc
from concourse.bass_utils import run_bass_kernel_spmd

N_CORES = 8
B_PC = 2
H = W = 512
NW = 64
NSLAB = 64
CH = 12
F32 = mybir.dt.float32
BF16 = mybir.dt.bfloat16
NPBF16 = ml_dtypes.bfloat16


def _rbloc(xa, c):
    if xa == 0:
        return 16 + c
    if xa == 9:
        return 18 + c
    return (xa - 1) * 2 + c


def _rloc(xa, ci):
    if xa == 0:
        return 96 + ci
    if xa == 9:
        return 108 + ci
    return (xa - 1) * 12 + ci


def _r0loc(ky, ci, xa):
    if ky == 0:
        if ci == 0:
            return xa
        if ci == 3:
            return 10 + xa
        return 20 + _rbloc(xa, ci - 1)
    if ky == 1:
        if ci == 0:
            return 40 + xa
        if ci == 3:
            return 50 + xa
        return 64 + _rbloc(xa, ci - 1)
    if ci == 0:
        return 84 + xa
    if ci == 3:
        return 94 + xa
    return 104 + _rbloc(xa, ci - 1)


def build_r0(mosaic):
    B = mosaic.shape[0]
    mp = np.zeros((B, 4, H + 2, W + 2), np.float32)
    mp[:, :, 1 : H + 1, 1 : W + 1] = mosaic
    r0 = np.zeros((B, 128, H, NW), np.float32)
    for ky in range(3):
        for ci in range(4):
            for xa in range(10):
                r0[:, _r0loc(ky, ci, xa)] = mp[:, ci, ky : ky + H, xa : xa + 8 * NW : 8]
    return r0


def build_w_l0(wt):
    W_ = np.zeros((128, 96), np.float32)
    for ky in range(3):
        for ci in range(4):
            for xa in range(10):
                for xo in range(8):
                    kx = xa - xo
                    if 0 <= kx <= 2:
                        for co in range(CH):
                            W_[_r0loc(ky, ci, xa), xo * 12 + co] = wt[co, ci, ky, kx]
    return W_


def build_w_int(wt):
    W_ = np.zeros((3, 120, 96), np.float32)
    for ky in range(3):
        for xa in range(10):
            for xo in range(8):
                kx = xa - xo
                if 0 <= kx <= 2:
                    k = _rloc(xa, 0)
                    W_[ky, k : k + 12, xo * 12 : xo * 12 + 12] = wt[:, :, ky, kx].T
    return W_


def build_w_sums():
    wse = np.zeros((96, 8), np.float32)
    wsep = np.zeros((96, 16), np.float32)
    wbc = np.zeros((8, 16), np.float32)
    for xo in range(8):
        for co in range(CH):
            wse[xo * 12 + co, xo] = 1.0
            wsep[xo * 12 + co, xo * 2 + (co >= 6)] = 1.0
        wbc[xo, xo * 2 : xo * 2 + 2] = 1.0
    return wse, wsep, wbc


def build_w_chroma(cw0):
    # chroma_pred = conv(rb) - conv(g) + green_add, all folded into:
    #   wg[ky] @ grb rows  (the -conv(g) part + green_add's g1/g0 picks)
    #   wchm  @ r0 rows    (the +conv(rb) part via r0's per-ky rb replicas
    #                       + green_add's m0/m3 picks)
    wg = np.zeros((3, 20, 48), np.float32)
    for ky in range(3):
        for xa in range(10):
            for xo in range(8):
                kx = xa - xo
                if 0 <= kx <= 2:
                    for co in range(6):
                        for d in range(2):
                            wg[ky, _rbloc(xa, d), xo * 6 + co] = -cw0[co, d, ky, kx]
    # green_add = [m0, g1, m3, m0, g0, m3]
    for xo in range(8):
        wg[1, _rbloc(xo + 1, 1), xo * 6 + 1] += 1.0
        wg[1, _rbloc(xo + 1, 0), xo * 6 + 4] += 1.0
    wchm = np.zeros((128, 48), np.float32)
    for xo in range(8):
        xa = xo + 1
        wchm[_r0loc(1, 0, xa), xo * 6 + 0] = 1.0
        wchm[_r0loc(1, 0, xa), xo * 6 + 3] = 1.0
        wchm[_r0loc(1, 3, xa), xo * 6 + 2] = 1.0
        wchm[_r0loc(1, 3, xa), xo * 6 + 5] = 1.0
    for ky in range(3):
        for xa in range(10):
            for xo in range(8):
                kx = xa - xo
                if 0 <= kx <= 2:
                    for co in range(6):
                        for d in range(2):
                            wchm[_r0loc(ky, d + 1, xa), xo * 6 + co] += cw0[
                                co, d, ky, kx
                            ]
    return wg, wchm


def assemble_output(mosaic, cp_dev, g_dev):
    B = mosaic.shape[0]
    cp_dev = np.asarray(cp_dev, np.float32)
    g_dev = np.asarray(g_dev, np.float32)
    cp = cp_dev.reshape(B, 8, 6, H, NW).transpose(0, 2, 3, 4, 1).reshape(B, 6, H, W)
    g = g_dev.reshape(B, 8, 2, H, NW).transpose(0, 2, 3, 4, 1).reshape(B, 2, H, W)
    m = mosaic
    out = np.empty((B, 3, 2 * H, 2 * W), np.float32)
    out[:, 0, 0::2, 0::2] = cp[:, 0]
    out[:, 0, 0::2, 1::2] = m[:, 1]
    out[:, 0, 1::2, 0::2] = cp[:, 1]
    out[:, 0, 1::2, 1::2] = cp[:, 2]
    out[:, 1, 0::2, 0::2] = m[:, 0]
    out[:, 1, 0::2, 1::2] = g[:, 0]
    out[:, 1, 1::2, 0::2] = g[:, 1]
    out[:, 1, 1::2, 1::2] = m[:, 3]
    out[:, 2, 0::2, 0::2] = cp[:, 3]
    out[:, 2, 0::2, 1::2] = cp[:, 4]
    out[:, 2, 1::2, 0::2] = m[:, 2]
    out[:, 2, 1::2, 1::2] = cp[:, 5]
    return out


# column offsets inside the packed [128, 1576] stationary tensor
_WOFF = {"wf0": 0, "ww0": 96, "wf1": 192, "wf2": 480, "ww1": 768, "ww2": 1056,
         "wse": 1344, "wsep": 1352, "wbc": 1368, "wg": 1384, "wchm": 1528}
_WCOLS = 1576


def pack_stationaries(st):
    wp = np.zeros((128, _WCOLS), np.float32)
    wp[:, 0:96] = st["wf0"]
    wp[:, 96:192] = st["ww0"]
    for nm in ("wf1", "wf2", "ww1", "ww2"):
        o = _WOFF[nm]
        for ky in range(3):
            wp[0:120, o + 96 * ky : o + 96 * (ky + 1)] = st[nm][ky]
    wp[0:96, 1344:1352] = st["wse"]
    wp[0:96, 1352:1368] = st["wsep"]
    wp[0:8, 1368:1384] = st["wbc"]
    for ky in range(3):
        wp[0:20, 1384 + 48 * ky : 1384 + 48 * (ky + 1)] = st["wg"][ky]
    wp[:, 1528:1576] = st["wchm"]
    return wp


def build_program():
    from contextlib import ExitStack

    nc = bacc.Bacc(
        "TRN2", target_bir_lowering=False, debug=False, num_devices=N_CORES
    )
    r0 = nc.declare_dram_parameter("r0", [B_PC, 128, H, NW], BF16, isOutput=False)
    wpack = nc.declare_dram_parameter("wpack", [128, _WCOLS], BF16, isOutput=False)
    out_cp = nc.declare_dram_parameter("out_cp", [B_PC, 48, H, NW], BF16, isOutput=True)
    out_g = nc.declare_dram_parameter("out_g", [B_PC, 16, H, NW], BF16, isOutput=True)

    Relu = mybir.ActivationFunctionType.Relu
    Exp = mybir.ActivationFunctionType.Exp
    Ln = mybir.ActivationFunctionType.Ln
    Copy = mybir.ActivationFunctionType.Copy
    NSTEPS = B_PC * NSLAB

    with tile.TileContext(nc) as tc, ExitStack() as ctx:
        const = ctx.enter_context(tc.tile_pool(name="const", bufs=1))
        r0pool = ctx.enter_context(tc.tile_pool(name="r0pool", bufs=6))
        p_rf1 = ctx.enter_context(tc.tile_pool(name="rf1", bufs=4))
        p_rw1 = ctx.enter_context(tc.tile_pool(name="rw1", bufs=4))
        p_rf2 = ctx.enter_context(tc.tile_pool(name="rf2", bufs=4))
        p_rw2 = ctx.enter_context(tc.tile_pool(name="rw2", bufs=4))
        p_grb = ctx.enter_context(tc.tile_pool(name="grb", bufs=4))
        p_act = ctx.enter_context(tc.tile_pool(name="acts", bufs=3))
        p_stg = ctx.enter_context(tc.tile_pool(name="stg", bufs=3))
        ps_mm = ctx.enter_context(tc.tile_pool(name="psmm", bufs=4, space="PSUM"))
        ps_sm = ctx.enter_context(tc.tile_pool(name="pssm", bufs=2, space="PSUM"))
        ps_cp = ctx.enter_context(tc.tile_pool(name="pscp", bufs=2, space="PSUM"))

        WC = const.tile([128, _WCOLS], BF16, tag="wpack_sb", name="wpack_sb")
        nc.sync.dma_start(out=WC[:], in_=wpack[:])
        sb = {
            "wf0": WC[:, 0:96],
            "ww0": WC[:, 96:192],
            "wse": WC[0:96, 1344:1352],
            "wsep": WC[0:96, 1352:1368],
            "wbc": WC[0:8, 1368:1384],
            "wchm": WC[:, 1528:1576],
        }

        def wky(nm, ky):
            o = _WOFF[nm]
            if nm == "wg":
                return WC[0:20, o + 48 * ky : o + 48 * (ky + 1)]
            return WC[0:120, o + 96 * ky : o + 96 * (ky + 1)]

        r0s, rf1, rw1, rf2, rw2, grb = {}, {}, {}, {}, {}, {}

        def get_rbuf(pool, dct, s):
            if s in dct or not (0 <= s < NSTEPS):
                return dct.get(s)
            t = pool.tile([120, 10, NW], BF16)
            dct[s] = t
            sl = s % NSLAB
            if sl == 0:
                nc.vector.memset(t[:, 0:1, :], 0.0)
            if sl == NSLAB - 1:
                nc.vector.memset(t[:, 9:10, :], 0.0)
            nc.vector.memset(t[96:120, :, 0:1], 0.0)
            nc.vector.memset(t[96:120, :, 63:64], 0.0)
            return t

        def get_grb(s):
            if s in grb or not (0 <= s < NSTEPS):
                return grb.get(s)
            t = p_grb.tile([20, 10, NW], BF16, name="g")
            grb[s] = t
            sl = s % NSLAB
            if sl == 0:
                nc.vector.memset(t[:, 0:1, :], 0.0)
            if sl == NSLAB - 1:
                nc.vector.memset(t[:, 9:10, :], 0.0)
            nc.vector.memset(t[:, :, 0:1], 0.0)
            nc.vector.memset(t[:, :, 63:64], 0.0)
            return t

        def conv_int(nm, rbuf):
            ps = ps_mm.tile([96, 8, NW], F32, tag="mm96", name="psc")
            for ky in range(3):
                nc.tensor.matmul(
                    ps[:],
                    wky(nm, ky),
                    rbuf[:, ky : ky + 8, :],
                    start=(ky == 0),
                    stop=(ky == 2),
                )
            return ps

        def evict(ps, dct, s, eng):
            # eng: ACT for the f-branch, DVE for the w-branch (load balance)
            sl = s % NSLAB

            def wr(out, in_):
                if eng == "act":
                    nc.scalar.activation(out=out, in_=in_, func=Relu)
                else:
                    nc.vector.tensor_relu(out=out, in_=in_)

            wr(dct[s][0:96, 1:9, :], ps[:])
            if sl < NSLAB - 1:
                wr(dct[s + 1][0:96, 0:1, :], ps[:, 7:8, :])
            if sl > 0:
                wr(dct[s - 1][0:96, 9:10, :], ps[:, 0:1, :])

        def strips(t):
            nc.sync.dma_start(out=t[96:108, :, 1:NW], in_=t[84:96, :, 0 : NW - 1])
            nc.sync.dma_start(out=t[108:120, :, 0 : NW - 1], in_=t[0:12, :, 1:NW])

        for T in range(NSTEPS + 3):
            s0 = T
            if 0 <= s0 < NSTEPS:
                img, sl = divmod(s0, NSLAB)
                y0 = sl * 8
                rt = r0pool.tile([128, 8, NW], BF16, name="rt")
                r0s[s0] = rt
                nc.sync.dma_start(out=rt[:], in_=r0[img, :, y0 : y0 + 8, :])
                get_rbuf(p_rf1, rf1, s0)
                get_rbuf(p_rf1, rf1, s0 + 1)
                get_rbuf(p_rw1, rw1, s0)
                get_rbuf(p_rw1, rw1, s0 + 1)
                psf = ps_mm.tile([96, 8, NW], F32, tag="mm96", name="psf0")
                nc.tensor.matmul(psf[:], sb["wf0"], rt[:], start=True, stop=True)
                evict(psf, rf1, s0, "act")
                psw = ps_mm.tile([96, 8, NW], F32, tag="mm96", name="psw0")
                nc.tensor.matmul(psw[:], sb["ww0"], rt[:], start=True, stop=True)
                evict(psw, rw1, s0, "dve")

            s1 = T - 1
            if 0 <= s1 < NSTEPS:
                strips(rf1[s1])
                strips(rw1[s1])
                get_rbuf(p_rf2, rf2, s1)
                get_rbuf(p_rf2, rf2, s1 + 1)
                get_rbuf(p_rw2, rw2, s1)
                get_rbuf(p_rw2, rw2, s1 + 1)
                evict(conv_int("wf1", rf1[s1]), rf2, s1, "act")
                evict(conv_int("ww1", rw1[s1]), rw2, s1, "dve")

            s2 = T - 2
            if 0 <= s2 < NSTEPS:
                strips(rf2[s2])
                strips(rw2[s2])
                psf = conv_int("wf2", rf2[s2])
                psw = conv_int("ww2", rw2[s2])
                # E = exp(relu(psw)) = max(exp(psw), 1)
                E0 = p_act.tile([96, 8, NW], BF16, tag="E0", name="E0")
                nc.scalar.activation(out=E0[:], in_=psw[:], func=Exp)
                E = p_act.tile([96, 8, NW], BF16, tag="E", name="E")
                nc.vector.tensor_scalar_max(E[:], E0[:], 1.0)
                # EP = relu(psf) * E = max(psf * E, 0)   (E > 0)
                EPt = p_act.tile([96, 8, NW], BF16, tag="EPt", name="EPt")
                nc.vector.tensor_mul(EPt[:], psf[:], E[:])
                EP = p_act.tile([96, 8, NW], BF16, tag="EP", name="EP")
                nc.vector.tensor_scalar_max(EP[:], EPt[:], 0.0)
                pse = ps_sm.tile([8, 8, NW], F32, tag="sm", name="pse")
                nc.tensor.matmul(pse[:], sb["wse"], E[:], start=True, stop=True)
                psep = ps_sm.tile([16, 8, NW], F32, tag="sm", name="psep")
                nc.tensor.matmul(psep[:], sb["wsep"], EP[:], start=True, stop=True)
                # 1/pse via exp(-ln(pse)); pse >= 12 so Ln is safe
                lnp = p_act.tile([8, 8, NW], F32, tag="lnp", name="lnp")
                nc.scalar.activation(out=lnp[:], in_=pse[:], func=Ln)
                rcp = p_act.tile([8, 8, NW], BF16, tag="rcp", name="rcp")
                nc.scalar.activation(out=rcp[:], in_=lnp[:], func=Exp, scale=-1.0)
                psbc = ps_sm.tile([16, 8, NW], F32, tag="sm", name="psbc")
                nc.tensor.matmul(psbc[:], sb["wbc"], rcp[:], start=True, stop=True)
                bcs = p_act.tile([16, 8, NW], BF16, tag="bcs", name="bcs")
                nc.vector.tensor_copy(out=bcs[:], in_=psbc[:])
                get_grb(s2)
                get_grb(s2 + 1)
                g = grb[s2]
                nc.vector.tensor_mul(g[0:16, 1:9, :], psep[:], bcs[:])
                sl = s2 % NSLAB
                if sl < NSLAB - 1:
                    nc.vector.tensor_copy(
                        out=grb[s2 + 1][0:16, 0:1, :], in_=g[0:16, 8:9, :]
                    )
                if sl > 0:
                    nc.vector.tensor_copy(
                        out=grb[s2 - 1][0:16, 9:10, :], in_=g[0:16, 1:2, :]
                    )

            s3 = T - 3
            if 0 <= s3 < NSTEPS:
                img, sl = divmod(s3, NSLAB)
                y0 = sl * 8
                g = grb[s3]
                nc.sync.dma_start(out=g[16:18, :, 1:NW], in_=g[14:16, :, 0 : NW - 1])
                nc.sync.dma_start(out=g[18:20, :, 0 : NW - 1], in_=g[0:2, :, 1:NW])
                rt = r0s[s3]
                pc = ps_cp.tile([48, 8, NW], F32, tag="cp", name="pc")
                for ky in range(3):
                    nc.tensor.matmul(
                        pc[:],
                        wky("wg", ky),
                        g[:, ky : ky + 8, :],
                        start=(ky == 0),
                        stop=False,
                    )
                nc.tensor.matmul(pc[:], sb["wchm"], rt[:], start=False, stop=True)
                stg = p_stg.tile([48, 8, NW], BF16, name="stg")
                nc.scalar.activation(out=stg[:], in_=pc[:], func=Copy)
                nc.sync.dma_start(out=out_cp[img, :, y0 : y0 + 8, :], in_=stg[:])
                nc.sync.dma_start(out=out_g[img, :, y0 : y0 + 8, :], in_=g[0:16, 1:9, :])
                for dct in (r0s, rf1, rw1, rf2, rw2, grb):
                    dct.pop(s3 - 2, None)

    nc.compile()
    return nc


_CACHE = {}


def kernel(mosaic, fw0, fw1, fw2, ww0, ww1, ww2, cw0, _trace=False):
    mosaic = np.asarray(mosaic, np.float32)
    r0_all = build_r0(mosaic).astype(NPBF16)

    stat = {
        "wf0": build_w_l0(np.asarray(fw0, np.float32)),
        "ww0": build_w_l0(np.asarray(ww0, np.float32)),
        "wf1": build_w_int(np.asarray(fw1, np.float32)),
        "wf2": build_w_int(np.asarray(fw2, np.float32)),
        "ww1": build_w_int(np.asarray(ww1, np.float32)),
        "ww2": build_w_int(np.asarray(ww2, np.float32)),
    }
    stat["wse"], stat["wsep"], stat["wbc"] = build_w_sums()
    stat["wg"], stat["wchm"] = build_w_chroma(np.asarray(cw0, np.float32))
    wpack = pack_stationaries(stat).astype(NPBF16)

    if "nc" not in _CACHE:
        _CACHE["nc"] = build_program()
    nc = _CACHE["nc"]

    in_maps = []
    for c in range(N_CORES):
        in_maps.append(
            {"r0": np.ascontiguousarray(r0_all[c * B_PC : (c + 1) * B_PC]),
             "wpack": wpack}
        )

    res = run_bass_kernel_spmd(nc, in_maps, list(range(N_CORES)), trace=_trace)
    outs = []
    for c in range(N_CORES):
        outs.append(
            assemble_output(
                mosaic[c * B_PC : (c + 1) * B_PC],
                res.results[c]["out_cp"],
                res.results[c]["out_g"],
            )
        )
    full = np.concatenate(outs, axis=0)
    if _trace:
        return full, res
    return full


# revision 12
# speedup vs baseline: 2.1967x; 1.6527x over previous
"""Trainium2 Bass kernel for BasicQuadRGBModel (quad-Bayer demosaic CNN).

v5: bf16 + ring buffers + batched/spread DMA issue.
  - activations live in ONE ring tensor RNG [120p, 4 slots, 32 rows, 64]:
    slot 0/1 = f/w layer-1 inputs, slot 2/3 = f/w layer-2 inputs. Slab s
    occupies rows (8s mod 32)..+7; convs read a 10-row window with the row
    halo coming from neighbouring slabs' rows already in the ring, so each
    PSUM eviction is ONE engine instruction (no neighbour-row writes).
  - conv = banded matmuls (K=120, M=96) per ky; ky=1 issued first covering
    all 8 out rows (start=True), ky=0/2 accumulate partial row ranges
    (image-edge rows dropped entirely: PSUM has_written handles it).
    Ring-wrap windows split into 2 accumulating matmuls.
  - x-halo strips: SBUF->SBUF DMAs using a flat-shifted AP (dst flat+1 =
    src flat) covering both f and w slots in one DMA; the row-boundary
    smear lands exactly on the zero-pad columns and is re-zeroed by tiny
    gpsimd memsets.
  - layer-0 im2col r0 (bf16, host-built) loaded 4 slabs per DMA; outputs
    staged in 64-row rings and shipped 32 rows per DMA.
  - softmax: E=max(exp(psw),1) (ACT exp + DVE max); EP=(psf max 0)*E in one
    DVE scalar_tensor_tensor; 1/sum via DVE reciprocal_approx_fast.
  - chroma conv(rb-g)+green_add folded host-side into wchm@r0 + wg@G.
  - DMA issue cost (~0.85us each on the issuing queue) spread: SP gets
    g-strips/rt/strips-A, gpsimd gets strips-B (swdge), ACT gets outputs.
  - host does layer-0 im2col and the final 2x2 pixel-shuffle.
"""

import sys

sys.path.insert(0, "/opt/trn_rl_repo")

import ml_dtypes
import numpy as np

import concourse.bass as bass
import concourse.mybir as mybir
import concourse.tile as tile
from concourse import bacc
from concourse.bass_utils import run_bass_kernel_spmd

N_CORES = 8
B_PC = 2
H = W = 512
NW = 64
NSLAB = 64
CH = 12
RR = 32    # activation ring rows (4 slabs)
RRG = 64   # g ring rows (8 slabs)
GP = 36    # g ring partitions: main [0:16), xa0 strip [32:34), xa9 [34:36)
F32 = mybir.dt.float32
BF16 = mybir.dt.bfloat16
NPBF16 = ml_dtypes.bfloat16


def _rbloc(xa, c):
    if xa == 0:
        return 16 + c
    if xa == 9:
        return 18 + c
    return (xa - 1) * 2 + c


def _rloc(xa, ci):
    if xa == 0:
        return 96 + ci
    if xa == 9:
        return 108 + ci
    return (xa - 1) * 12 + ci


def _r0loc(ky, ci, xa):
    if ky == 0:
        if ci == 0:
            return xa
        if ci == 3:
            return 10 + xa
        return 20 + _rbloc(xa, ci - 1)
    if ky == 1:
        if ci == 0:
            return 40 + xa
        if ci == 3:
            return 50 + xa
        return 64 + _rbloc(xa, ci - 1)
    if ci == 0:
        return 84 + xa
    if ci == 3:
        return 94 + xa
    return 104 + _rbloc(xa, ci - 1)


def build_r0(mosaic):
    B = mosaic.shape[0]
    mp = np.zeros((B, 4, H + 2, W + 2), np.float32)
    mp[:, :, 1 : H + 1, 1 : W + 1] = mosaic
    r0 = np.zeros((B, 128, H, NW), np.float32)
    for ky in range(3):
        for ci in range(4):
            for xa in range(10):
                r0[:, _r0loc(ky, ci, xa)] = mp[:, ci, ky : ky + H, xa : xa + 8 * NW : 8]
    return r0


def build_w_l0(wt):
    W_ = np.zeros((128, 96), np.float32)
    for ky in range(3):
        for ci in range(4):
            for xa in range(10):
                for xo in range(8):
                    kx = xa - xo
                    if 0 <= kx <= 2:
                        for co in range(CH):
                            W_[_r0loc(ky, ci, xa), xo * 12 + co] = wt[co, ci, ky, kx]
    return W_


def build_w_int(wt):
    W_ = np.zeros((3, 120, 96), np.float32)
    for ky in range(3):
        for xa in range(10):
            for xo in range(8):
                kx = xa - xo
                if 0 <= kx <= 2:
                    k = _rloc(xa, 0)
                    W_[ky, k : k + 12, xo * 12 : xo * 12 + 12] = wt[:, :, ky, kx].T
    return W_


def build_w_sums():
    wse = np.zeros((96, 8), np.float32)
    wsep = np.zeros((96, 16), np.float32)
    wbc = np.zeros((8, 16), np.float32)
    for xo in range(8):
        for co in range(CH):
            wse[xo * 12 + co, xo] = 1.0
            wsep[xo * 12 + co, xo * 2 + (co >= 6)] = 1.0
        wbc[xo, xo * 2 : xo * 2 + 2] = 1.0
    return wse, wsep, wbc


def _gloc(xa, c):
    if xa == 0:
        return 32 + c
    if xa == 9:
        return 34 + c
    return (xa - 1) * 2 + c


def build_w_chroma(cw0):
    # chroma_pred = conv(rb) - conv(g) + green_add, folded into:
    #   wg[ky] @ G rows   (the -conv(g) part + green_add's g1/g0 picks)
    #   wchm  @ r0 rows   (the +conv(rb) part via r0's per-ky rb replicas
    #                      + green_add's m0/m3 picks)
    wg = np.zeros((3, 36, 48), np.float32)
    for ky in range(3):
        for xa in range(10):
            for xo in range(8):
                kx = xa - xo
                if 0 <= kx <= 2:
                    for co in range(6):
                        for d in range(2):
                            wg[ky, _gloc(xa, d), xo * 6 + co] = -cw0[co, d, ky, kx]
    for xo in range(8):
        wg[1, _gloc(xo + 1, 1), xo * 6 + 1] += 1.0
        wg[1, _gloc(xo + 1, 0), xo * 6 + 4] += 1.0
    wchm = np.zeros((128, 48), np.float32)
    for xo in range(8):
        xa = xo + 1
        wchm[_r0loc(1, 0, xa), xo * 6 + 0] = 1.0
        wchm[_r0loc(1, 0, xa), xo * 6 + 3] = 1.0
        wchm[_r0loc(1, 3, xa), xo * 6 + 2] = 1.0
        wchm[_r0loc(1, 3, xa), xo * 6 + 5] = 1.0
    for ky in range(3):
        for xa in range(10):
            for xo in range(8):
                kx = xa - xo
                if 0 <= kx <= 2:
                    for co in range(6):
                        for d in range(2):
                            wchm[_r0loc(ky, d + 1, xa), xo * 6 + co] += cw0[
                                co, d, ky, kx
                            ]
    return wg, wchm


def assemble_output(mosaic, cp_dev, g_dev):
    B = mosaic.shape[0]
    cp_dev = np.asarray(cp_dev, np.float32)
    g_dev = np.asarray(g_dev, np.float32)
    cp = cp_dev.reshape(B, 8, 6, H, NW).transpose(0, 2, 3, 4, 1).reshape(B, 6, H, W)
    g = g_dev.reshape(B, 8, 2, H, NW).transpose(0, 2, 3, 4, 1).reshape(B, 2, H, W)
    m = mosaic
    out = np.empty((B, 3, 2 * H, 2 * W), np.float32)
    out[:, 0, 0::2, 0::2] = cp[:, 0]
    out[:, 0, 0::2, 1::2] = m[:, 1]
    out[:, 0, 1::2, 0::2] = cp[:, 1]
    out[:, 0, 1::2, 1::2] = cp[:, 2]
    out[:, 1, 0::2, 0::2] = m[:, 0]
    out[:, 1, 0::2, 1::2] = g[:, 0]
    out[:, 1, 1::2, 0::2] = g[:, 1]
    out[:, 1, 1::2, 1::2] = m[:, 3]
    out[:, 2, 0::2, 0::2] = cp[:, 3]
    out[:, 2, 0::2, 1::2] = cp[:, 4]
    out[:, 2, 1::2, 0::2] = m[:, 2]
    out[:, 2, 1::2, 1::2] = cp[:, 5]
    return out


# column offsets inside the packed [128, 1576] stationary tensor
_WOFF = {"wf0": 0, "ww0": 96, "wf1": 192, "wf2": 480, "ww1": 768, "ww2": 1056,
         "wse": 1344, "wsep": 1352, "wbc": 1368, "wg": 1384, "wchm": 1528}
_WCOLS = 1576


def pack_stationaries(st):
    wp = np.zeros((128, _WCOLS), np.float32)
    wp[:, 0:96] = st["wf0"]
    wp[:, 96:192] = st["ww0"]
    for nm in ("wf1", "wf2", "ww1", "ww2"):
        o = _WOFF[nm]
        for ky in range(3):
            wp[0:120, o + 96 * ky : o + 96 * (ky + 1)] = st[nm][ky]
    wp[0:96, 1344:1352] = st["wse"]
    wp[0:96, 1352:1368] = st["wsep"]
    wp[0:8, 1368:1384] = st["wbc"]
    for ky in range(3):
        wp[0:36, 1384 + 48 * ky : 1384 + 48 * (ky + 1)] = st["wg"][ky]
    wp[:, 1528:1576] = st["wchm"]
    return wp


def _row_pieces(base, n, ring):
    """Split ring-row window [base, base+n) (mod ring) into linear pieces."""
    base %= ring
    if base + n <= ring:
        return [(base, n)]
    return [(base, ring - base), (0, n - (ring - base))]


def build_program():
    from contextlib import ExitStack

    nc = bacc.Bacc(
        "TRN2", target_bir_lowering=False, debug=False, num_devices=N_CORES
    )
    r0 = nc.declare_dram_parameter("r0", [B_PC, 128, H, NW], BF16, isOutput=False)
    wpack = nc.declare_dram_parameter("wpack", [128, _WCOLS], BF16, isOutput=False)
    out_cp = nc.declare_dram_parameter("out_cp", [B_PC, 48, H, NW], BF16, isOutput=True)
    out_g = nc.declare_dram_parameter("out_g", [B_PC, 16, H, NW], BF16, isOutput=True)

    Relu = mybir.ActivationFunctionType.Relu
    Exp = mybir.ActivationFunctionType.Exp
    Copy = mybir.ActivationFunctionType.Copy
    MAX = mybir.AluOpType.max
    MULT = mybir.AluOpType.mult
    NSTEPS = B_PC * NSLAB

    with tile.TileContext(nc) as tc, ExitStack() as ctx:
        const = ctx.enter_context(tc.tile_pool(name="const", bufs=1))
        r0pool = ctx.enter_context(tc.tile_pool(name="r0pool", bufs=4))
        p_act = ctx.enter_context(tc.tile_pool(name="acts", bufs=3))
        ps_mm = ctx.enter_context(tc.tile_pool(name="psmm", bufs=4, space="PSUM"))
        ps_sm = ctx.enter_context(tc.tile_pool(name="pssm", bufs=2, space="PSUM"))
        ps_cp = ctx.enter_context(tc.tile_pool(name="pscp", bufs=2, space="PSUM"))

        WC = const.tile([128, _WCOLS], BF16, tag="wpack_sb", name="wpack_sb")
        nc.sync.dma_start(out=WC[:], in_=wpack[:])
        RNG = const.tile([120, 4, RR, NW], BF16, tag="ring", name="ring")
        G = const.tile([GP, RRG, NW], BF16, tag="gring", name="gring")
        STG = const.tile([48, RRG, NW], BF16, tag="stg", name="stg")

        nc.vector.memset(G[0:GP, :, :], 0.0)
        # one-time zero of the x-pad columns inside the RNG halo strips
        # (engine memsets can't start at partition 108; DMA can)
        ZZ = const.tile([16, 128], BF16, tag="zz", name="zz")
        nc.vector.memset(ZZ[:], 0.0)
        nc.sync.dma_start(out=RNG[96:108, :, :, 0:1], in_=ZZ[0:12, 0:128])
        nc.sync.dma_start(out=RNG[108:120, :, :, 63:64], in_=ZZ[0:12, 0:128])

        sb = {
            "wf0": WC[:, 0:96],
            "ww0": WC[:, 96:192],
            "wse": WC[0:96, 1344:1352],
            "wsep": WC[0:96, 1352:1368],
            "wbc": WC[0:8, 1368:1384],
            "wchm": WC[:, 1528:1576],
        }

        def wky(nm, ky):
            o = _WOFF[nm]
            if nm == "wg":
                return WC[0:36, o + 48 * ky : o + 48 * (ky + 1)]
            return WC[0:120, o + 96 * ky : o + 96 * (ky + 1)]

        def conv_pieces(s, ring):
            """(ky, in_row, out_row, nrows) pieces; ky=1 first (always full).

            out row r sums input rows 8s+r+ky-1; image-edge taps (row -1 /
            row H) are dropped entirely (PSUM has_written handles partial
            accumulation); ring-wrap windows split into two pieces."""
            sl = s % NSLAB
            out = [(1, (8 * s) % ring, 0, 8)]
            if sl == 0:
                out.append((0, (8 * s) % ring, 1, 7))
            else:
                ro = 0
                for rb, n in _row_pieces(8 * s - 1, 8, ring):
                    out.append((0, rb, ro, n))
                    ro += n
            if sl == NSLAB - 1:
                out.append((2, (8 * s + 1) % ring, 0, 7))
            else:
                ro = 0
                for rb, n in _row_pieces(8 * s + 1, 8, ring):
                    out.append((2, rb, ro, n))
                    ro += n
            return out

        def conv_ring(nm, slot, s):
            ps = ps_mm.tile([96, 8, NW], F32, tag="mm96", name="psc")
            pieces = conv_pieces(s, RR)
            for i, (ky, ri, ro, n) in enumerate(pieces):
                nc.tensor.matmul(
                    ps[:, ro : ro + n, :],
                    wky(nm, ky),
                    RNG[0:120, slot, ri : ri + n, :],
                    start=(i == 0),
                    stop=(i == len(pieces) - 1),
                )
            return ps

        def evict(ps, slot, s, eng):
            rows = (8 * s) % RR
            out = RNG[0:96, slot, rows : rows + 8, :]
            if eng == "act":
                nc.scalar.activation(out=out, in_=ps[:], func=Relu)
            else:
                nc.vector.tensor_relu(out=out, in_=ps[:])

        def ring_strips(b, nrows, slots, eng):
            """x-halo strips for ring-row window [b, b+nrows) per slot.
            Plain 2D shifted copies: pad cols (xa0 col 0 / xa9 col 63) are
            never touched, so the one-time init zeros persist."""
            for slot in slots:
                for rb, n in _row_pieces(b, nrows, RR):
                    eng.dma_start(
                        out=RNG[96:108, slot, rb : rb + n, 1:NW],
                        in_=RNG[84:96, slot, rb : rb + n, 0 : NW - 1],
                    )
                    eng.dma_start(
                        out=RNG[108:120, slot, rb : rb + n, 0 : NW - 1],
                        in_=RNG[0:12, slot, rb : rb + n, 1:NW],
                    )

        def g_strips(b, nrows, eng):
            for rb, n in _row_pieces(b, nrows, RRG):
                eng.dma_start(
                    out=G[32:34, rb : rb + n, 1:NW],
                    in_=G[14:16, rb : rb + n, 0 : NW - 1],
                )
                eng.dma_start(
                    out=G[34:36, rb : rb + n, 0 : NW - 1],
                    in_=G[0:2, rb : rb + n, 1:NW],
                )

        rt4s = {}

        for T in range(NSTEPS + 7):
            # stage s0: layer-0 convs from r0 (4-slab granule loads)
            s0 = T
            if 0 <= s0 < NSTEPS:
                if s0 % 4 == 0:
                    img = s0 // NSLAB
                    y0 = (s0 % NSLAB) * 8
                    rt4 = r0pool.tile([128, 32, NW], BF16, name="rt4")
                    rt4s[s0 // 4] = rt4
                    nc.sync.dma_start(out=rt4[:], in_=r0[img, :, y0 : y0 + 32, :])
                rt4 = rt4s[s0 // 4]
                k = s0 % 4
                psf = ps_mm.tile([96, 8, NW], F32, tag="mm96", name="psf0")
                nc.tensor.matmul(
                    psf[:], sb["wf0"], rt4[:, 8 * k : 8 * k + 8, :],
                    start=True, stop=True,
                )
                evict(psf, 0, s0, "act")
                psw = ps_mm.tile([96, 8, NW], F32, tag="mm96", name="psw0")
                nc.tensor.matmul(
                    psw[:], sb["ww0"], rt4[:, 8 * k : 8 * k + 8, :],
                    start=True, stop=True,
                )
                evict(psw, 1, s0, "dve")

            # strips for L1 readers at T and T+1 (slabs T-2, T-1)
            if T % 2 == 0 and 0 <= T - 2 < NSTEPS:
                ring_strips((8 * (T - 2)) % RR, 17, (0, 1), nc.sync)

            # stage s2: layer-1 convs
            s2 = T - 2
            if 0 <= s2 < NSTEPS:
                evict(conv_ring("wf1", 0, s2), 2, s2, "act")
                evict(conv_ring("ww1", 1, s2), 3, s2, "dve")

            # strips for L2 readers at T and T+1 (slabs T-4, T-3)
            if T % 2 == 0 and 0 <= T - 4 < NSTEPS:
                ring_strips((8 * (T - 4)) % RR, 17, (2, 3), nc.gpsimd)

            # stage s4: layer-2 convs + softmax + g
            s4 = T - 4
            if 0 <= s4 < NSTEPS:
                psf = conv_ring("wf2", 2, s4)
                psw = conv_ring("ww2", 3, s4)
                # E = exp(relu(psw)) = max(exp(psw), 1)
                E0 = p_act.tile([96, 8, NW], BF16, tag="E0", name="E0")
                nc.scalar.activation(out=E0[:], in_=psw[:], func=Exp)
                E = p_act.tile([96, 8, NW], BF16, tag="E", name="E")
                nc.vector.tensor_scalar_max(E[:], E0[:], 1.0)
                # EP = relu(psf) * E in one fused DVE op
                EP = p_act.tile([96, 8, NW], BF16, tag="EP", name="EP")
                nc.vector.scalar_tensor_tensor(EP[:], psf[:], 0.0, E[:], MAX, MULT)
                pse = ps_sm.tile([8, 8, NW], F32, tag="sm", name="pse")
                nc.tensor.matmul(pse[:], sb["wse"], E[:], start=True, stop=True)
                psep = ps_sm.tile([16, 8, NW], F32, tag="sm", name="psep")
                nc.tensor.matmul(psep[:], sb["wsep"], EP[:], start=True, stop=True)
                rcpf = p_act.tile([8, 8, NW], F32, tag="rcpf", name="rcpf")
                nc.vector.reciprocal_approx_fast(out=rcpf[:], in_=pse[:])
                rcp = p_act.tile([8, 8, NW], BF16, tag="rcp", name="rcp")
                nc.vector.tensor_copy(out=rcp[:], in_=rcpf[:])
                psbc = ps_sm.tile([16, 8, NW], F32, tag="sm", name="psbc")
                nc.tensor.matmul(psbc[:], sb["wbc"], rcp[:], start=True, stop=True)
                bcs = p_act.tile([16, 8, NW], BF16, tag="bcs", name="bcs")
                nc.scalar.activation(out=bcs[:], in_=psbc[:], func=Copy)
                gr = (8 * s4) % RRG
                nc.vector.tensor_mul(G[0:16, gr : gr + 8, :], psep[:], bcs[:])

            # g strips for chroma readers at T and T+1 (slabs T-6, T-5)
            if T % 2 == 0 and 0 <= T - 6 < NSTEPS:
                g_strips((8 * (T - 6)) % RRG, 17, nc.gpsimd)

            # stage s6: chroma + staging + batched outputs
            s6 = T - 6
            if 0 <= s6 < NSTEPS:
                pc = ps_cp.tile([48, 8, NW], F32, tag="cp", name="pc")
                rt4 = rt4s[s6 // 4]
                k = s6 % 4
                nc.tensor.matmul(
                    pc[:], sb["wchm"], rt4[:, 8 * k : 8 * k + 8, :],
                    start=True, stop=False,
                )
                pieces = conv_pieces(s6, RRG)
                for i, (ky, ri, ro, n) in enumerate(pieces):
                    nc.tensor.matmul(
                        pc[:, ro : ro + n, :],
                        wky("wg", ky),
                        G[0:36, ri : ri + n, :],
                        start=False,
                        stop=(i == len(pieces) - 1),
                    )
                sg = (8 * s6) % RRG
                nc.scalar.activation(out=STG[:, sg : sg + 8, :], in_=pc[:], func=Copy)
                if s6 % 4 == 3:
                    img = s6 // NSLAB
                    y0 = ((s6 - 3) % NSLAB) * 8
                    blk = (8 * (s6 - 3)) % RRG
                    nc.scalar.dma_start(
                        out=out_cp[img, :, y0 : y0 + 32, :],
                        in_=STG[0:48, blk : blk + 32, :],
                    )
                    nc.scalar.dma_start(
                        out=out_g[img, :, y0 : y0 + 32, :],
                        in_=G[0:16, blk : blk + 32, :],
                    )
                rt4s.pop(s6 // 4 - 3, None)

    nc.compile()
    return nc


_CACHE = {}


def kernel(mosaic, fw0, fw1, fw2, ww0, ww1, ww2, cw0, _trace=False):
    mosaic = np.asarray(mosaic, np.float32)
    r0_all = build_r0(mosaic).astype(NPBF16)

    stat = {
        "wf0": build_w_l0(np.asarray(fw0, np.float32)),
        "ww0": build_w_l0(np.asarray(ww0, np.float32)),
        "wf1": build_w_int(np.asarray(fw1, np.float32)),
        "wf2": build_w_int(np.asarray(fw2, np.float32)),
        "ww1": build_w_int(np.asarray(ww1, np.float32)),
        "ww2": build_w_int(np.asarray(ww2, np.float32)),
    }
    stat["wse"], stat["wsep"], stat["wbc"] = build_w_sums()
    stat["wg"], stat["wchm"] = build_w_chroma(np.asarray(cw0, np.float32))
    wpack = pack_stationaries(stat).astype(NPBF16)

    if "nc" not in _CACHE:
        _CACHE["nc"] = build_program()
    nc = _CACHE["nc"]

    in_maps = []
    for c in range(N_CORES):
        in_maps.append(
            {"r0": np.ascontiguousarray(r0_all[c * B_PC : (c + 1) * B_PC]),
             "wpack": wpack}
        )

    res = run_bass_kernel_spmd(nc, in_maps, list(range(N_CORES)), trace=_trace)
    outs = []
    for c in range(N_CORES):
        outs.append(
            assemble_output(
                mosaic[c * B_PC : (c + 1) * B_PC],
                res.results[c]["out_cp"],
                res.results[c]["out_g"],
            )
        )
    full = np.concatenate(outs, axis=0)
    if _trace:
        return full, res
    return full


# revision 13
# speedup vs baseline: 2.6683x; 1.2147x over previous
"""Trainium2 Bass kernel for BasicQuadRGBModel (quad-Bayer demosaic CNN).

v5: bf16 + ring buffers + batched/spread DMA issue.
  - activations live in ONE ring tensor RNG [120p, 4 slots, 32 rows, 64]:
    slot 0/1 = f/w layer-1 inputs, slot 2/3 = f/w layer-2 inputs. Slab s
    occupies rows (8s mod 32)..+7; convs read a 10-row window with the row
    halo coming from neighbouring slabs' rows already in the ring, so each
    PSUM eviction is ONE engine instruction (no neighbour-row writes).
  - conv = banded matmuls (K=120, M=96) per ky; ky=1 issued first covering
    all 8 out rows (start=True), ky=0/2 accumulate partial row ranges
    (image-edge rows dropped entirely: PSUM has_written handles it).
    Ring-wrap windows split into 2 accumulating matmuls.
  - x-halo strips: SBUF->SBUF DMAs using a flat-shifted AP (dst flat+1 =
    src flat) covering both f and w slots in one DMA; the row-boundary
    smear lands exactly on the zero-pad columns and is re-zeroed by tiny
    gpsimd memsets.
  - layer-0 im2col r0 (bf16, host-built) loaded 4 slabs per DMA; outputs
    staged in 64-row rings and shipped 32 rows per DMA.
  - softmax: E=max(exp(psw),1) (ACT exp + DVE max); EP=(psf max 0)*E in one
    DVE scalar_tensor_tensor; 1/sum via DVE reciprocal_approx_fast.
  - chroma conv(rb-g)+green_add folded host-side into wchm@r0 + wg@G.
  - DMA issue cost (~0.85us each on the issuing queue) spread: SP gets
    g-strips/rt/strips-A, gpsimd gets strips-B (swdge), ACT gets outputs.
  - host does layer-0 im2col and the final 2x2 pixel-shuffle.
"""

import sys

sys.path.insert(0, "/opt/trn_rl_repo")

import ml_dtypes
import numpy as np

import concourse.bass as bass
import concourse.mybir as mybir
import concourse.tile as tile
from concourse import bacc
from concourse.bass_utils import run_bass_kernel_spmd

N_CORES = 8
B_PC = 2
H = W = 512
NW = 64
NSLAB = 64
CH = 12
RR = 32    # activation ring rows (4 slabs)
RRG = 64   # g ring rows (8 slabs)
GP = 36    # g ring partitions: main [0:16), xa0 strip [32:34), xa9 [34:36)
F32 = mybir.dt.float32
BF16 = mybir.dt.bfloat16
NPBF16 = ml_dtypes.bfloat16


def _rbloc(xa, c):
    if xa == 0:
        return 16 + c
    if xa == 9:
        return 18 + c
    return (xa - 1) * 2 + c


def _rloc(xa, ci):
    if xa == 0:
        return 96 + ci
    if xa == 9:
        return 108 + ci
    return (xa - 1) * 12 + ci


def _r0loc(ky, ci, xa):
    if ky == 0:
        if ci == 0:
            return xa
        if ci == 3:
            return 10 + xa
        return 20 + _rbloc(xa, ci - 1)
    if ky == 1:
        if ci == 0:
            return 40 + xa
        if ci == 3:
            return 50 + xa
        return 64 + _rbloc(xa, ci - 1)
    if ci == 0:
        return 84 + xa
    if ci == 3:
        return 94 + xa
    return 104 + _rbloc(xa, ci - 1)


def build_r0(mosaic):
    B = mosaic.shape[0]
    mp = np.zeros((B, 4, H + 2, W + 2), np.float32)
    mp[:, :, 1 : H + 1, 1 : W + 1] = mosaic
    r0 = np.zeros((B, 128, H, NW), np.float32)
    for ky in range(3):
        for ci in range(4):
            for xa in range(10):
                r0[:, _r0loc(ky, ci, xa)] = mp[:, ci, ky : ky + H, xa : xa + 8 * NW : 8]
    return r0


def build_w_l0(wt):
    W_ = np.zeros((128, 96), np.float32)
    for ky in range(3):
        for ci in range(4):
            for xa in range(10):
                for xo in range(8):
                    kx = xa - xo
                    if 0 <= kx <= 2:
                        for co in range(CH):
                            W_[_r0loc(ky, ci, xa), xo * 12 + co] = wt[co, ci, ky, kx]
    return W_


def build_w_int(wt):
    W_ = np.zeros((3, 120, 96), np.float32)
    for ky in range(3):
        for xa in range(10):
            for xo in range(8):
                kx = xa - xo
                if 0 <= kx <= 2:
                    k = _rloc(xa, 0)
                    W_[ky, k : k + 12, xo * 12 : xo * 12 + 12] = wt[:, :, ky, kx].T
    return W_


def build_w_sums():
    wse = np.zeros((96, 8), np.float32)
    wsep = np.zeros((96, 16), np.float32)
    wbc = np.zeros((8, 16), np.float32)
    for xo in range(8):
        for co in range(CH):
            wse[xo * 12 + co, xo] = 1.0
            wsep[xo * 12 + co, xo * 2 + (co >= 6)] = 1.0
        wbc[xo, xo * 2 : xo * 2 + 2] = 1.0
    return wse, wsep, wbc


def _gloc(xa, c):
    if xa == 0:
        return 32 + c
    if xa == 9:
        return 34 + c
    return (xa - 1) * 2 + c


def build_w_chroma(cw0):
    # chroma_pred = conv(rb) - conv(g) + green_add, folded into:
    #   wg[ky] @ G rows   (the -conv(g) part + green_add's g1/g0 picks)
    #   wchm  @ r0 rows   (the +conv(rb) part via r0's per-ky rb replicas
    #                      + green_add's m0/m3 picks)
    wg = np.zeros((3, 36, 48), np.float32)
    for ky in range(3):
        for xa in range(10):
            for xo in range(8):
                kx = xa - xo
                if 0 <= kx <= 2:
                    for co in range(6):
                        for d in range(2):
                            wg[ky, _gloc(xa, d), xo * 6 + co] = -cw0[co, d, ky, kx]
    for xo in range(8):
        wg[1, _gloc(xo + 1, 1), xo * 6 + 1] += 1.0
        wg[1, _gloc(xo + 1, 0), xo * 6 + 4] += 1.0
    wchm = np.zeros((128, 48), np.float32)
    for xo in range(8):
        xa = xo + 1
        wchm[_r0loc(1, 0, xa), xo * 6 + 0] = 1.0
        wchm[_r0loc(1, 0, xa), xo * 6 + 3] = 1.0
        wchm[_r0loc(1, 3, xa), xo * 6 + 2] = 1.0
        wchm[_r0loc(1, 3, xa), xo * 6 + 5] = 1.0
    for ky in range(3):
        for xa in range(10):
            for xo in range(8):
                kx = xa - xo
                if 0 <= kx <= 2:
                    for co in range(6):
                        for d in range(2):
                            wchm[_r0loc(ky, d + 1, xa), xo * 6 + co] += cw0[
                                co, d, ky, kx
                            ]
    return wg, wchm


def assemble_output(mosaic, cp_dev, g_dev):
    B = mosaic.shape[0]
    cp_dev = np.asarray(cp_dev, np.float32)
    g_dev = np.asarray(g_dev, np.float32)
    cp = cp_dev.reshape(B, 8, 6, H, NW).transpose(0, 2, 3, 4, 1).reshape(B, 6, H, W)
    g = g_dev.reshape(B, 8, 2, H, NW).transpose(0, 2, 3, 4, 1).reshape(B, 2, H, W)
    m = mosaic
    out = np.empty((B, 3, 2 * H, 2 * W), np.float32)
    out[:, 0, 0::2, 0::2] = cp[:, 0]
    out[:, 0, 0::2, 1::2] = m[:, 1]
    out[:, 0, 1::2, 0::2] = cp[:, 1]
    out[:, 0, 1::2, 1::2] = cp[:, 2]
    out[:, 1, 0::2, 0::2] = m[:, 0]
    out[:, 1, 0::2, 1::2] = g[:, 0]
    out[:, 1, 1::2, 0::2] = g[:, 1]
    out[:, 1, 1::2, 1::2] = m[:, 3]
    out[:, 2, 0::2, 0::2] = cp[:, 3]
    out[:, 2, 0::2, 1::2] = cp[:, 4]
    out[:, 2, 1::2, 0::2] = m[:, 2]
    out[:, 2, 1::2, 1::2] = cp[:, 5]
    return out


# column offsets inside the packed [128, 1576] stationary tensor
_WOFF = {"wf0": 0, "ww0": 96, "wf1": 192, "wf2": 480, "ww1": 768, "ww2": 1056,
         "wse": 1344, "wsep": 1352, "wbc": 1368, "wg": 1384, "wchm": 1528}
_WCOLS = 1576


def pack_stationaries(st):
    wp = np.zeros((128, _WCOLS), np.float32)
    wp[:, 0:96] = st["wf0"]
    wp[:, 96:192] = st["ww0"]
    for nm in ("wf1", "wf2", "ww1", "ww2"):
        o = _WOFF[nm]
        for ky in range(3):
            wp[0:120, o + 96 * ky : o + 96 * (ky + 1)] = st[nm][ky]
    wp[0:96, 1344:1352] = st["wse"]
    wp[0:96, 1352:1368] = st["wsep"]
    wp[0:8, 1368:1384] = st["wbc"]
    for ky in range(3):
        wp[0:36, 1384 + 48 * ky : 1384 + 48 * (ky + 1)] = st["wg"][ky]
    wp[:, 1528:1576] = st["wchm"]
    return wp


def _row_pieces(base, n, ring):
    """Split ring-row window [base, base+n) (mod ring) into linear pieces."""
    base %= ring
    if base + n <= ring:
        return [(base, n)]
    return [(base, ring - base), (0, n - (ring - base))]


def build_program():
    from contextlib import ExitStack

    nc = bacc.Bacc(
        "TRN2", target_bir_lowering=False, debug=False, num_devices=N_CORES
    )
    r0 = nc.declare_dram_parameter("r0", [B_PC, 128, H, NW], BF16, isOutput=False)
    wpack = nc.declare_dram_parameter("wpack", [128, _WCOLS], BF16, isOutput=False)
    out_cp = nc.declare_dram_parameter("out_cp", [B_PC, 48, H, NW], BF16, isOutput=True)
    out_g = nc.declare_dram_parameter("out_g", [B_PC, 16, H, NW], BF16, isOutput=True)

    Relu = mybir.ActivationFunctionType.Relu
    Exp = mybir.ActivationFunctionType.Exp
    Copy = mybir.ActivationFunctionType.Copy
    MAX = mybir.AluOpType.max
    MULT = mybir.AluOpType.mult
    NSTEPS = B_PC * NSLAB

    with tile.TileContext(nc) as tc, ExitStack() as ctx:
        const = ctx.enter_context(tc.tile_pool(name="const", bufs=1))
        r0pool = ctx.enter_context(tc.tile_pool(name="r0pool", bufs=4))
        p_act = ctx.enter_context(tc.tile_pool(name="acts", bufs=3))
        ps_mm = ctx.enter_context(tc.tile_pool(name="psmm", bufs=4, space="PSUM"))
        ps_sm = ctx.enter_context(tc.tile_pool(name="pssm", bufs=2, space="PSUM"))
        ps_cp = ctx.enter_context(tc.tile_pool(name="pscp", bufs=2, space="PSUM"))

        WC = const.tile([128, _WCOLS], BF16, tag="wpack_sb", name="wpack_sb")
        nc.sync.dma_start(out=WC[:], in_=wpack[:])
        RNG = const.tile([120, 4, RR, NW], BF16, tag="ring", name="ring")
        G = const.tile([GP, RRG, NW], BF16, tag="gring", name="gring")
        STG = const.tile([48, RRG, NW], BF16, tag="stg", name="stg")

        nc.vector.memset(G[0:GP, :, :], 0.0)
        # one-time zero of the x-pad columns inside the RNG halo strips
        # (engine memsets can't start at partition 108; DMA can)
        ZZ = const.tile([16, 128], BF16, tag="zz", name="zz")
        nc.vector.memset(ZZ[:], 0.0)
        nc.sync.dma_start(out=RNG[96:108, :, :, 0:1], in_=ZZ[0:12, 0:128])
        nc.sync.dma_start(out=RNG[108:120, :, :, 63:64], in_=ZZ[0:12, 0:128])

        sb = {
            "wf0": WC[:, 0:96],
            "ww0": WC[:, 96:192],
            "wse": WC[0:96, 1344:1352],
            "wsep": WC[0:96, 1352:1368],
            "wbc": WC[0:8, 1368:1384],
            "wchm": WC[:, 1528:1576],
        }

        def wky(nm, ky):
            o = _WOFF[nm]
            if nm == "wg":
                return WC[0:36, o + 48 * ky : o + 48 * (ky + 1)]
            return WC[0:120, o + 96 * ky : o + 96 * (ky + 1)]

        def conv_pieces(s, ring):
            """(ky, in_row, out_row, nrows) pieces; ky=1 first (always full).

            out row r sums input rows 8s+r+ky-1; image-edge taps (row -1 /
            row H) are dropped entirely (PSUM has_written handles partial
            accumulation); ring-wrap windows split into two pieces."""
            sl = s % NSLAB
            out = [(1, (8 * s) % ring, 0, 8)]
            if sl == 0:
                out.append((0, (8 * s) % ring, 1, 7))
            else:
                ro = 0
                for rb, n in _row_pieces(8 * s - 1, 8, ring):
                    out.append((0, rb, ro, n))
                    ro += n
            if sl == NSLAB - 1:
                out.append((2, (8 * s + 1) % ring, 0, 7))
            else:
                ro = 0
                for rb, n in _row_pieces(8 * s + 1, 8, ring):
                    out.append((2, rb, ro, n))
                    ro += n
            return out

        def conv_ring(nm, slot, s):
            ps = ps_mm.tile([96, 8, NW], F32, tag="mm96", name="psc")
            pieces = conv_pieces(s, RR)
            for i, (ky, ri, ro, n) in enumerate(pieces):
                nc.tensor.matmul(
                    ps[:, ro : ro + n, :],
                    wky(nm, ky),
                    RNG[0:120, slot, ri : ri + n, :],
                    start=(i == 0),
                    stop=(i == len(pieces) - 1),
                )
            return ps

        def evict(ps, slot, s, eng):
            rows = (8 * s) % RR
            out = RNG[0:96, slot, rows : rows + 8, :]
            if eng == "act":
                nc.scalar.activation(out=out, in_=ps[:], func=Relu)
            else:
                nc.vector.tensor_relu(out=out, in_=ps[:])

        def ring_strips(b, nrows, slots, eng):
            """x-halo strips for ring-row window [b, b+nrows) per slot.
            Plain 2D shifted copies: pad cols (xa0 col 0 / xa9 col 63) are
            never touched, so the one-time init zeros persist."""
            for slot in slots:
                for rb, n in _row_pieces(b, nrows, RR):
                    eng.dma_start(
                        out=RNG[96:108, slot, rb : rb + n, 1:NW],
                        in_=RNG[84:96, slot, rb : rb + n, 0 : NW - 1],
                    )
                    eng.dma_start(
                        out=RNG[108:120, slot, rb : rb + n, 0 : NW - 1],
                        in_=RNG[0:12, slot, rb : rb + n, 1:NW],
                    )

        def g_strips(b, nrows, eng):
            for rb, n in _row_pieces(b, nrows, RRG):
                eng.dma_start(
                    out=G[32:34, rb : rb + n, 1:NW],
                    in_=G[14:16, rb : rb + n, 0 : NW - 1],
                )
                eng.dma_start(
                    out=G[34:36, rb : rb + n, 0 : NW - 1],
                    in_=G[0:2, rb : rb + n, 1:NW],
                )

        rt4s = {}
        es, eps, rcps, smas = {}, {}, {}, {}

        for T in range(NSTEPS + 11):
            # g strips for chroma readers at T and T+1 (slabs T-10, T-9);
            # all content is >=2 iterations old -> gpsimd issues immediately
            if T % 2 == 0 and 0 <= T - 10 < NSTEPS:
                g_strips((8 * (T - 10)) % RRG, 17, nc.gpsimd)

            # stage sD = T-5: softmax sums for slab sD (E/EP made last iter)
            sD = T - 5
            if 0 <= sD < NSTEPS:
                sm = ps_sm.tile([128, 8, NW], F32, tag="smA", name="smA")
                smas[sD] = sm
                nc.tensor.matmul(sm[0:8], sb["wse"], es[sD][:], start=True, stop=True)
                nc.tensor.matmul(
                    sm[32:48], sb["wsep"], eps[sD][:], start=True, stop=True
                )
                rcpf = p_act.tile([8, 8, NW], F32, tag="rcpf", name="rcpf")
                nc.vector.reciprocal_approx_fast(out=rcpf[:], in_=sm[0:8])
                rcp = p_act.tile([8, 8, NW], BF16, tag="rcp", name="rcp")
                nc.vector.tensor_copy(out=rcp[:], in_=rcpf[:])
                rcps[sD] = rcp

            # stage sE = T-6: softmax broadcast + g rows for slab sE
            sE = T - 6
            if 0 <= sE < NSTEPS:
                sm = smas[sE]
                nc.tensor.matmul(
                    sm[64:80], sb["wbc"], rcps[sE][:], start=True, stop=True
                )
                bcs = p_act.tile([16, 8, NW], BF16, tag="bcs", name="bcs")
                nc.scalar.activation(out=bcs[:], in_=sm[64:80], func=Copy)
                gr = (8 * sE) % RRG
                nc.vector.tensor_mul(G[0:16, gr : gr + 8, :], sm[32:48], bcs[:])

            # stage sA = T: layer-0 convs from r0 (4-slab granule loads)
            s0 = T
            if 0 <= s0 < NSTEPS:
                if s0 % 4 == 0:
                    img = s0 // NSLAB
                    y0 = (s0 % NSLAB) * 8
                    rt4 = r0pool.tile([128, 32, NW], BF16, name="rt4")
                    rt4s[s0 // 4] = rt4
                    nc.sync.dma_start(out=rt4[:], in_=r0[img, :, y0 : y0 + 32, :])
                rt4 = rt4s[s0 // 4]
                k = s0 % 4
                psf = ps_mm.tile([96, 8, NW], F32, tag="mm96", name="psf0")
                nc.tensor.matmul(
                    psf[:], sb["wf0"], rt4[:, 8 * k : 8 * k + 8, :],
                    start=True, stop=True,
                )
                evict(psf, 0, s0, "act")
                psw = ps_mm.tile([96, 8, NW], F32, tag="mm96", name="psw0")
                nc.tensor.matmul(
                    psw[:], sb["ww0"], rt4[:, 8 * k : 8 * k + 8, :],
                    start=True, stop=True,
                )
                evict(psw, 1, s0, "dve")

            # strips for L1 readers at T and T+1 (slabs T-2, T-1)
            if T % 2 == 0 and 0 <= T - 2 < NSTEPS:
                ring_strips((8 * (T - 2)) % RR, 17, (0, 1), nc.sync)

            # stage sB = T-2: layer-1 convs
            s2 = T - 2
            if 0 <= s2 < NSTEPS:
                evict(conv_ring("wf1", 0, s2), 2, s2, "act")
                evict(conv_ring("ww1", 1, s2), 3, s2, "dve")

            # strips for L2 readers at T and T+1 (slabs T-4, T-3)
            if T % 2 == 0 and 0 <= T - 4 < NSTEPS:
                ring_strips((8 * (T - 4)) % RR, 17, (2, 3), nc.gpsimd)

            # stage sC = T-4: layer-2 convs + E/EP (consumed next iteration)
            s4 = T - 4
            if 0 <= s4 < NSTEPS:
                psf = conv_ring("wf2", 2, s4)
                psw = conv_ring("ww2", 3, s4)
                E0 = p_act.tile([96, 8, NW], BF16, tag="E0", name="E0")
                nc.scalar.activation(out=E0[:], in_=psw[:], func=Exp)
                E = p_act.tile([96, 8, NW], BF16, tag="E", name="E")
                nc.vector.tensor_scalar_max(E[:], E0[:], 1.0)
                es[s4] = E
                EP = p_act.tile([96, 8, NW], BF16, tag="EP", name="EP")
                nc.vector.scalar_tensor_tensor(EP[:], psf[:], 0.0, E[:], MAX, MULT)
                eps[s4] = EP

            # stage sF = T-10: chroma + staging + batched outputs
            s6 = T - 10
            if 0 <= s6 < NSTEPS:
                pc = ps_cp.tile([48, 8, NW], F32, tag="cp", name="pc")
                rt4 = rt4s[s6 // 4]
                k = s6 % 4
                nc.tensor.matmul(
                    pc[:], sb["wchm"], rt4[:, 8 * k : 8 * k + 8, :],
                    start=True, stop=False,
                )
                pieces = conv_pieces(s6, RRG)
                for i, (ky, ri, ro, n) in enumerate(pieces):
                    nc.tensor.matmul(
                        pc[:, ro : ro + n, :],
                        wky("wg", ky),
                        G[0:36, ri : ri + n, :],
                        start=False,
                        stop=(i == len(pieces) - 1),
                    )
                sg = (8 * s6) % RRG
                nc.scalar.activation(out=STG[:, sg : sg + 8, :], in_=pc[:], func=Copy)
                if s6 % 4 == 3:
                    img = s6 // NSLAB
                    y0 = ((s6 - 3) % NSLAB) * 8
                    blk = (8 * (s6 - 3)) % RRG
                    nc.scalar.dma_start(
                        out=out_cp[img, :, y0 : y0 + 32, :],
                        in_=STG[0:48, blk : blk + 32, :],
                    )
                    nc.scalar.dma_start(
                        out=out_g[img, :, y0 : y0 + 32, :],
                        in_=G[0:16, blk : blk + 32, :],
                    )
                rt4s.pop(s6 // 4 - 3, None)
                for dd in (es, eps, rcps, smas):
                    dd.pop(s6, None)

    nc.compile()
    return nc


_CACHE = {}


def kernel(mosaic, fw0, fw1, fw2, ww0, ww1, ww2, cw0, _trace=False):
    mosaic = np.asarray(mosaic, np.float32)
    r0_all = build_r0(mosaic).astype(NPBF16)

    stat = {
        "wf0": build_w_l0(np.asarray(fw0, np.float32)),
        "ww0": build_w_l0(np.asarray(ww0, np.float32)),
        "wf1": build_w_int(np.asarray(fw1, np.float32)),
        "wf2": build_w_int(np.asarray(fw2, np.float32)),
        "ww1": build_w_int(np.asarray(ww1, np.float32)),
        "ww2": build_w_int(np.asarray(ww2, np.float32)),
    }
    stat["wse"], stat["wsep"], stat["wbc"] = build_w_sums()
    stat["wg"], stat["wchm"] = build_w_chroma(np.asarray(cw0, np.float32))
    wpack = pack_stationaries(stat).astype(NPBF16)

    if "nc" not in _CACHE:
        _CACHE["nc"] = build_program()
    nc = _CACHE["nc"]

    in_maps = []
    for c in range(N_CORES):
        in_maps.append(
            {"r0": np.ascontiguousarray(r0_all[c * B_PC : (c + 1) * B_PC]),
             "wpack": wpack}
        )

    res = run_bass_kernel_spmd(nc, in_maps, list(range(N_CORES)), trace=_trace)
    outs = []
    for c in range(N_CORES):
        outs.append(
            assemble_output(
                mosaic[c * B_PC : (c + 1) * B_PC],
                res.results[c]["out_cp"],
                res.results[c]["out_g"],
            )
        )
    full = np.concatenate(outs, axis=0)
    if _trace:
        return full, res
    return full


# revision 15
# speedup vs baseline: 2.7396x; 1.0268x over previous
"""Trainium2 Bass kernel for BasicQuadRGBModel (quad-Bayer demosaic CNN).

v5: bf16 + ring buffers + batched/spread DMA issue.
  - activations live in ONE ring tensor RNG [120p, 4 slots, 32 rows, 64]:
    slot 0/1 = f/w layer-1 inputs, slot 2/3 = f/w layer-2 inputs. Slab s
    occupies rows (8s mod 32)..+7; convs read a 10-row window with the row
    halo coming from neighbouring slabs' rows already in the ring, so each
    PSUM eviction is ONE engine instruction (no neighbour-row writes).
  - conv = banded matmuls (K=120, M=96) per ky; ky=1 issued first covering
    all 8 out rows (start=True), ky=0/2 accumulate partial row ranges
    (image-edge rows dropped entirely: PSUM has_written handles it).
    Ring-wrap windows split into 2 accumulating matmuls.
  - x-halo strips: SBUF->SBUF DMAs using a flat-shifted AP (dst flat+1 =
    src flat) covering both f and w slots in one DMA; the row-boundary
    smear lands exactly on the zero-pad columns and is re-zeroed by tiny
    gpsimd memsets.
  - layer-0 im2col r0 (bf16, host-built) loaded 4 slabs per DMA; outputs
    staged in 64-row rings and shipped 32 rows per DMA.
  - softmax: E=max(exp(psw),1) (ACT exp + DVE max); EP=(psf max 0)*E in one
    DVE scalar_tensor_tensor; 1/sum via DVE reciprocal_approx_fast.
  - chroma conv(rb-g)+green_add folded host-side into wchm@r0 + wg@G.
  - DMA issue cost (~0.85us each on the issuing queue) spread: SP gets
    g-strips/rt/strips-A, gpsimd gets strips-B (swdge), ACT gets outputs.
  - host does layer-0 im2col and the final 2x2 pixel-shuffle.
"""

import sys

sys.path.insert(0, "/opt/trn_rl_repo")

import ml_dtypes
import numpy as np

import concourse.bass as bass
import concourse.mybir as mybir
import concourse.tile as tile
from concourse import bacc
from concourse.bass_utils import run_bass_kernel_spmd

N_CORES = 8
B_PC = 2
H = W = 512
NW = 64
NSLAB = 64
CH = 12
RR = 128   # activation ring rows (16 slabs)
RRG = 128  # g ring rows (16 slabs)
GP = 36    # g ring partitions: main [0:16), xa0 strip [32:34), xa9 [34:36)
F32 = mybir.dt.float32
BF16 = mybir.dt.bfloat16
NPBF16 = ml_dtypes.bfloat16


def _rbloc(xa, c):
    if xa == 0:
        return 16 + c
    if xa == 9:
        return 18 + c
    return (xa - 1) * 2 + c


def _rloc(xa, ci):
    if xa == 0:
        return 96 + ci
    if xa == 9:
        return 108 + ci
    return (xa - 1) * 12 + ci


def _r0loc(ky, ci, xa):
    if ky == 0:
        if ci == 0:
            return xa
        if ci == 3:
            return 10 + xa
        return 20 + _rbloc(xa, ci - 1)
    if ky == 1:
        if ci == 0:
            return 40 + xa
        if ci == 3:
            return 50 + xa
        return 64 + _rbloc(xa, ci - 1)
    if ci == 0:
        return 84 + xa
    if ci == 3:
        return 94 + xa
    return 104 + _rbloc(xa, ci - 1)


def build_r0(mosaic):
    B = mosaic.shape[0]
    mp = np.zeros((B, 4, H + 2, W + 2), np.float32)
    mp[:, :, 1 : H + 1, 1 : W + 1] = mosaic
    r0 = np.zeros((B, 128, H, NW), np.float32)
    for ky in range(3):
        for ci in range(4):
            for xa in range(10):
                r0[:, _r0loc(ky, ci, xa)] = mp[:, ci, ky : ky + H, xa : xa + 8 * NW : 8]
    return r0


def build_w_l0(wt):
    W_ = np.zeros((128, 96), np.float32)
    for ky in range(3):
        for ci in range(4):
            for xa in range(10):
                for xo in range(8):
                    kx = xa - xo
                    if 0 <= kx <= 2:
                        for co in range(CH):
                            W_[_r0loc(ky, ci, xa), xo * 12 + co] = wt[co, ci, ky, kx]
    return W_


def build_w_int(wt):
    W_ = np.zeros((3, 120, 96), np.float32)
    for ky in range(3):
        for xa in range(10):
            for xo in range(8):
                kx = xa - xo
                if 0 <= kx <= 2:
                    k = _rloc(xa, 0)
                    W_[ky, k : k + 12, xo * 12 : xo * 12 + 12] = wt[:, :, ky, kx].T
    return W_


def build_w_sums():
    wse = np.zeros((96, 8), np.float32)
    wsep = np.zeros((96, 16), np.float32)
    wbc = np.zeros((8, 16), np.float32)
    for xo in range(8):
        for co in range(CH):
            wse[xo * 12 + co, xo] = 1.0
            wsep[xo * 12 + co, xo * 2 + (co >= 6)] = 1.0
        wbc[xo, xo * 2 : xo * 2 + 2] = 1.0
    return wse, wsep, wbc


def _gloc(xa, c):
    if xa == 0:
        return 32 + c
    if xa == 9:
        return 34 + c
    return (xa - 1) * 2 + c


def build_w_chroma(cw0):
    # chroma_pred = conv(rb) - conv(g) + green_add, folded into:
    #   wg[ky] @ G rows   (the -conv(g) part + green_add's g1/g0 picks)
    #   wchm  @ r0 rows   (the +conv(rb) part via r0's per-ky rb replicas
    #                      + green_add's m0/m3 picks)
    wg = np.zeros((3, 36, 48), np.float32)
    for ky in range(3):
        for xa in range(10):
            for xo in range(8):
                kx = xa - xo
                if 0 <= kx <= 2:
                    for co in range(6):
                        for d in range(2):
                            wg[ky, _gloc(xa, d), xo * 6 + co] = -cw0[co, d, ky, kx]
    for xo in range(8):
        wg[1, _gloc(xo + 1, 1), xo * 6 + 1] += 1.0
        wg[1, _gloc(xo + 1, 0), xo * 6 + 4] += 1.0
    wchm = np.zeros((128, 48), np.float32)
    for xo in range(8):
        xa = xo + 1
        wchm[_r0loc(1, 0, xa), xo * 6 + 0] = 1.0
        wchm[_r0loc(1, 0, xa), xo * 6 + 3] = 1.0
        wchm[_r0loc(1, 3, xa), xo * 6 + 2] = 1.0
        wchm[_r0loc(1, 3, xa), xo * 6 + 5] = 1.0
    for ky in range(3):
        for xa in range(10):
            for xo in range(8):
                kx = xa - xo
                if 0 <= kx <= 2:
                    for co in range(6):
                        for d in range(2):
                            wchm[_r0loc(ky, d + 1, xa), xo * 6 + co] += cw0[
                                co, d, ky, kx
                            ]
    return wg, wchm


def assemble_output(mosaic, cp_dev, g_dev):
    B = mosaic.shape[0]
    cp_dev = np.asarray(cp_dev, np.float32)
    g_dev = np.asarray(g_dev, np.float32)
    cp = cp_dev.reshape(B, 8, 6, H, NW).transpose(0, 2, 3, 4, 1).reshape(B, 6, H, W)
    g = g_dev.reshape(B, 8, 2, H, NW).transpose(0, 2, 3, 4, 1).reshape(B, 2, H, W)
    m = mosaic
    out = np.empty((B, 3, 2 * H, 2 * W), np.float32)
    out[:, 0, 0::2, 0::2] = cp[:, 0]
    out[:, 0, 0::2, 1::2] = m[:, 1]
    out[:, 0, 1::2, 0::2] = cp[:, 1]
    out[:, 0, 1::2, 1::2] = cp[:, 2]
    out[:, 1, 0::2, 0::2] = m[:, 0]
    out[:, 1, 0::2, 1::2] = g[:, 0]
    out[:, 1, 1::2, 0::2] = g[:, 1]
    out[:, 1, 1::2, 1::2] = m[:, 3]
    out[:, 2, 0::2, 0::2] = cp[:, 3]
    out[:, 2, 0::2, 1::2] = cp[:, 4]
    out[:, 2, 1::2, 0::2] = m[:, 2]
    out[:, 2, 1::2, 1::2] = cp[:, 5]
    return out


# column offsets inside the packed [128, 1576] stationary tensor
_WOFF = {"wf0": 0, "ww0": 96, "wf1": 192, "wf2": 480, "ww1": 768, "ww2": 1056,
         "wse": 1344, "wsep": 1352, "wbc": 1368, "wg": 1384, "wchm": 1528}
_WCOLS = 1576


def pack_stationaries(st):
    wp = np.zeros((128, _WCOLS), np.float32)
    wp[:, 0:96] = st["wf0"]
    wp[:, 96:192] = st["ww0"]
    for nm in ("wf1", "wf2", "ww1", "ww2"):
        o = _WOFF[nm]
        for ky in range(3):
            wp[0:120, o + 96 * ky : o + 96 * (ky + 1)] = st[nm][ky]
    wp[0:96, 1344:1352] = st["wse"]
    wp[0:96, 1352:1368] = st["wsep"]
    wp[0:8, 1368:1384] = st["wbc"]
    for ky in range(3):
        wp[0:36, 1384 + 48 * ky : 1384 + 48 * (ky + 1)] = st["wg"][ky]
    wp[:, 1528:1576] = st["wchm"]
    return wp


def _row_pieces(base, n, ring):
    """Split ring-row window [base, base+n) (mod ring) into linear pieces."""
    base %= ring
    if base + n <= ring:
        return [(base, n)]
    return [(base, ring - base), (0, n - (ring - base))]


def build_program():
    from contextlib import ExitStack

    nc = bacc.Bacc(
        "TRN2", target_bir_lowering=False, debug=False, num_devices=N_CORES
    )
    r0 = nc.declare_dram_parameter("r0", [B_PC, 128, H, NW], BF16, isOutput=False)
    wpack = nc.declare_dram_parameter("wpack", [128, _WCOLS], BF16, isOutput=False)
    out_cp = nc.declare_dram_parameter("out_cp", [B_PC, 48, H, NW], BF16, isOutput=True)
    out_g = nc.declare_dram_parameter("out_g", [B_PC, 16, H, NW], BF16, isOutput=True)

    Relu = mybir.ActivationFunctionType.Relu
    Exp = mybir.ActivationFunctionType.Exp
    Copy = mybir.ActivationFunctionType.Copy
    MAX = mybir.AluOpType.max
    MULT = mybir.AluOpType.mult
    NSTEPS = B_PC * NSLAB

    with tile.TileContext(nc) as tc, ExitStack() as ctx:
        const = ctx.enter_context(tc.tile_pool(name="const", bufs=1))
        r0pool = ctx.enter_context(tc.tile_pool(name="r0pool", bufs=4))
        p_act = ctx.enter_context(tc.tile_pool(name="acts", bufs=3))
        ps_mm = ctx.enter_context(tc.tile_pool(name="psmm", bufs=4, space="PSUM"))
        ps_sm = ctx.enter_context(tc.tile_pool(name="pssm", bufs=2, space="PSUM"))
        ps_cp = ctx.enter_context(tc.tile_pool(name="pscp", bufs=2, space="PSUM"))

        WC = const.tile([128, _WCOLS], BF16, tag="wpack_sb", name="wpack_sb")
        nc.sync.dma_start(out=WC[:], in_=wpack[:])
        RNG = const.tile([120, 4, RR, NW], BF16, tag="ring", name="ring")
        G = const.tile([GP, RRG, NW], BF16, tag="gring", name="gring")
        STG = const.tile([48, RRG, NW], BF16, tag="stg", name="stg")

        nc.vector.memset(G[0:GP, :, :], 0.0)
        # one-time zero of the x-pad columns inside the RNG halo strips
        # (engine memsets can't start at partition 108; DMA can)
        ZZ = const.tile([16, 4 * RR], BF16, tag="zz", name="zz")
        nc.vector.memset(ZZ[:], 0.0)
        nc.sync.dma_start(out=RNG[96:108, :, :, 0:1], in_=ZZ[0:12, :])
        nc.sync.dma_start(out=RNG[108:120, :, :, 63:64], in_=ZZ[0:12, :])

        sb = {
            "wf0": WC[:, 0:96],
            "ww0": WC[:, 96:192],
            "wse": WC[0:96, 1344:1352],
            "wsep": WC[0:96, 1352:1368],
            "wbc": WC[0:8, 1368:1384],
            "wchm": WC[:, 1528:1576],
        }

        def wky(nm, ky):
            o = _WOFF[nm]
            if nm == "wg":
                return WC[0:36, o + 48 * ky : o + 48 * (ky + 1)]
            return WC[0:120, o + 96 * ky : o + 96 * (ky + 1)]

        def conv_pieces(s, ring):
            """(ky, in_row, out_row, nrows) pieces; ky=1 first (always full).

            out row r sums input rows 8s+r+ky-1; image-edge taps (row -1 /
            row H) are dropped entirely (PSUM has_written handles partial
            accumulation); ring-wrap windows split into two pieces."""
            sl = s % NSLAB
            out = [(1, (8 * s) % ring, 0, 8)]
            if sl == 0:
                out.append((0, (8 * s) % ring, 1, 7))
            else:
                ro = 0
                for rb, n in _row_pieces(8 * s - 1, 8, ring):
                    out.append((0, rb, ro, n))
                    ro += n
            if sl == NSLAB - 1:
                out.append((2, (8 * s + 1) % ring, 0, 7))
            else:
                ro = 0
                for rb, n in _row_pieces(8 * s + 1, 8, ring):
                    out.append((2, rb, ro, n))
                    ro += n
            return out

        def conv_ring(nm, slot, s):
            ps = ps_mm.tile([96, 8, NW], F32, tag="mm96", name="psc")
            pieces = conv_pieces(s, RR)
            for i, (ky, ri, ro, n) in enumerate(pieces):
                nc.tensor.matmul(
                    ps[:, ro : ro + n, :],
                    wky(nm, ky),
                    RNG[0:120, slot, ri : ri + n, :],
                    start=(i == 0),
                    stop=(i == len(pieces) - 1),
                )
            return ps

        def evict(ps, slot, s, eng):
            rows = (8 * s) % RR
            out = RNG[0:96, slot, rows : rows + 8, :]
            if eng == "act":
                nc.scalar.activation(out=out, in_=ps[:], func=Relu)
            else:
                nc.vector.tensor_relu(out=out, in_=ps[:])

        def ring_strips(b, nrows, slots, eng):
            """x-halo strips for ring-row window [b, b+nrows) per slot.
            Plain 2D shifted copies: pad cols (xa0 col 0 / xa9 col 63) are
            never touched, so the one-time init zeros persist."""
            for slot in slots:
                for rb, n in _row_pieces(b, nrows, RR):
                    eng.dma_start(
                        out=RNG[96:108, slot, rb : rb + n, 1:NW],
                        in_=RNG[84:96, slot, rb : rb + n, 0 : NW - 1],
                    )
                    eng.dma_start(
                        out=RNG[108:120, slot, rb : rb + n, 0 : NW - 1],
                        in_=RNG[0:12, slot, rb : rb + n, 1:NW],
                    )

        def g_strips(b, nrows, eng):
            for rb, n in _row_pieces(b, nrows, RRG):
                eng.dma_start(
                    out=G[32:34, rb : rb + n, 1:NW],
                    in_=G[14:16, rb : rb + n, 0 : NW - 1],
                )
                eng.dma_start(
                    out=G[34:36, rb : rb + n, 0 : NW - 1],
                    in_=G[0:2, rb : rb + n, 1:NW],
                )

        rt4s = {}
        es, eps, rcps, smas = {}, {}, {}, {}

        for T in range(NSTEPS + 17):
            # g strips for chroma readers at T..T+3 (slabs T-16..T-13);
            # content is >=2 iterations old -> issues immediately
            if T % 4 == 0 and 0 <= T - 16 < NSTEPS:
                g_strips((8 * (T - 16)) % RRG, 33, nc.gpsimd)

            # stage sD = T-9: softmax sums for slab sD (E/EP made last iter)
            sD = T - 9
            if 0 <= sD < NSTEPS:
                sm = ps_sm.tile([128, 8, NW], F32, tag="smA", name="smA")
                smas[sD] = sm
                nc.tensor.matmul(sm[0:8], sb["wse"], es[sD][:], start=True, stop=True)
                nc.tensor.matmul(
                    sm[32:48], sb["wsep"], eps[sD][:], start=True, stop=True
                )
                rcpf = p_act.tile([8, 8, NW], F32, tag="rcpf", name="rcpf")
                nc.vector.reciprocal_approx_fast(out=rcpf[:], in_=sm[0:8])
                rcp = p_act.tile([8, 8, NW], BF16, tag="rcp", name="rcp")
                nc.vector.tensor_copy(out=rcp[:], in_=rcpf[:])
                rcps[sD] = rcp

            # stage sE = T-10: softmax broadcast + g rows for slab sE
            sE = T - 10
            if 0 <= sE < NSTEPS:
                sm = smas[sE]
                nc.tensor.matmul(
                    sm[64:80], sb["wbc"], rcps[sE][:], start=True, stop=True
                )
                bcs = p_act.tile([16, 8, NW], BF16, tag="bcs", name="bcs")
                nc.scalar.activation(out=bcs[:], in_=sm[64:80], func=Copy)
                gr = (8 * sE) % RRG
                nc.vector.tensor_mul(G[0:16, gr : gr + 8, :], sm[32:48], bcs[:])

            # stage sA = T: layer-0 convs from r0 (8-slab granule loads)
            s0 = T
            if 0 <= s0 < NSTEPS:
                if s0 % 8 == 0:
                    img = s0 // NSLAB
                    y0 = (s0 % NSLAB) * 8
                    rt4 = r0pool.tile([128, 64, NW], BF16, name="rt4")
                    rt4s[s0 // 8] = rt4
                    nc.sync.dma_start(out=rt4[:], in_=r0[img, :, y0 : y0 + 64, :])
                rt4 = rt4s[s0 // 8]
                k = s0 % 8
                psf = ps_mm.tile([96, 8, NW], F32, tag="mm96", name="psf0")
                nc.tensor.matmul(
                    psf[:], sb["wf0"], rt4[:, 8 * k : 8 * k + 8, :],
                    start=True, stop=True,
                )
                evict(psf, 0, s0, "act")
                psw = ps_mm.tile([96, 8, NW], F32, tag="mm96", name="psw0")
                nc.tensor.matmul(
                    psw[:], sb["ww0"], rt4[:, 8 * k : 8 * k + 8, :],
                    start=True, stop=True,
                )
                evict(psw, 1, s0, "dve")

            # strips for L1 readers at T..T+3 (slabs T-4..T-1)
            if T % 4 == 0 and 0 <= T - 4 < NSTEPS:
                ring_strips((8 * (T - 4)) % RR, 33, (0, 1), nc.sync)

            # stage sB = T-4: layer-1 convs
            s2 = T - 4
            if 0 <= s2 < NSTEPS:
                evict(conv_ring("wf1", 0, s2), 2, s2, "act")
                evict(conv_ring("ww1", 1, s2), 3, s2, "dve")

            # strips for L2 readers at T..T+3 (slabs T-8..T-5)
            if T % 4 == 0 and 0 <= T - 8 < NSTEPS:
                ring_strips((8 * (T - 8)) % RR, 33, (2, 3), nc.gpsimd)

            # stage sC = T-8: layer-2 convs + E/EP (consumed next iteration)
            s4 = T - 8
            if 0 <= s4 < NSTEPS:
                psf = conv_ring("wf2", 2, s4)
                psw = conv_ring("ww2", 3, s4)
                E0 = p_act.tile([96, 8, NW], BF16, tag="E0", name="E0")
                nc.scalar.activation(out=E0[:], in_=psw[:], func=Exp)
                E = p_act.tile([96, 8, NW], BF16, tag="E", name="E")
                nc.vector.tensor_scalar_max(E[:], E0[:], 1.0)
                es[s4] = E
                EP = p_act.tile([96, 8, NW], BF16, tag="EP", name="EP")
                nc.vector.scalar_tensor_tensor(EP[:], psf[:], 0.0, E[:], MAX, MULT)
                eps[s4] = EP

            # stage sF = T-16: chroma + staging + batched outputs
            s6 = T - 16
            if 0 <= s6 < NSTEPS:
                pc = ps_cp.tile([48, 8, NW], F32, tag="cp", name="pc")
                rt4 = rt4s[s6 // 8]
                k = s6 % 8
                nc.tensor.matmul(
                    pc[:], sb["wchm"], rt4[:, 8 * k : 8 * k + 8, :],
                    start=True, stop=False,
                )
                pieces = conv_pieces(s6, RRG)
                for i, (ky, ri, ro, n) in enumerate(pieces):
                    nc.tensor.matmul(
                        pc[:, ro : ro + n, :],
                        wky("wg", ky),
                        G[0:36, ri : ri + n, :],
                        start=False,
                        stop=(i == len(pieces) - 1),
                    )
                sg = (8 * s6) % RRG
                nc.scalar.activation(out=STG[:, sg : sg + 8, :], in_=pc[:], func=Copy)
                if s6 % 8 == 7:
                    img = s6 // NSLAB
                    y0 = ((s6 - 7) % NSLAB) * 8
                    blk = (8 * (s6 - 7)) % RRG
                    nc.scalar.dma_start(
                        out=out_cp[img, :, y0 : y0 + 64, :],
                        in_=STG[0:48, blk : blk + 64, :],
                    )
                    nc.scalar.dma_start(
                        out=out_g[img, :, y0 : y0 + 64, :],
                        in_=G[0:16, blk : blk + 64, :],
                    )
                rt4s.pop(s6 // 8 - 3, None)
                for dd in (es, eps, rcps, smas):
                    dd.pop(s6, None)

    nc.compile()
    return nc


_CACHE = {}


def kernel(mosaic, fw0, fw1, fw2, ww0, ww1, ww2, cw0, _trace=False):
    mosaic = np.asarray(mosaic, np.float32)
    r0_all = build_r0(mosaic).astype(NPBF16)

    stat = {
        "wf0": build_w_l0(np.asarray(fw0, np.float32)),
        "ww0": build_w_l0(np.asarray(ww0, np.float32)),
        "wf1": build_w_int(np.asarray(fw1, np.float32)),
        "wf2": build_w_int(np.asarray(fw2, np.float32)),
        "ww1": build_w_int(np.asarray(ww1, np.float32)),
        "ww2": build_w_int(np.asarray(ww2, np.float32)),
    }
    stat["wse"], stat["wsep"], stat["wbc"] = build_w_sums()
    stat["wg"], stat["wchm"] = build_w_chroma(np.asarray(cw0, np.float32))
    wpack = pack_stationaries(stat).astype(NPBF16)

    if "nc" not in _CACHE:
        _CACHE["nc"] = build_program()
    nc = _CACHE["nc"]

    in_maps = []
    for c in range(N_CORES):
        in_maps.append(
            {"r0": np.ascontiguousarray(r0_all[c * B_PC : (c + 1) * B_PC]),
             "wpack": wpack}
        )

    res = run_bass_kernel_spmd(nc, in_maps, list(range(N_CORES)), trace=_trace)
    outs = []
    for c in range(N_CORES):
        outs.append(
            assemble_output(
                mosaic[c * B_PC : (c + 1) * B_PC],
                res.results[c]["out_cp"],
                res.results[c]["out_g"],
            )
        )
    full = np.concatenate(outs, axis=0)
    if _trace:
        return full, res
    return full


# revision 17
# speedup vs baseline: 2.7839x; 1.0162x over previous
"""Trainium2 Bass kernel for BasicQuadRGBModel (quad-Bayer demosaic CNN).

v5: bf16 + ring buffers + batched/spread DMA issue.
  - activations live in ONE ring tensor RNG [120p, 4 slots, 32 rows, 64]:
    slot 0/1 = f/w layer-1 inputs, slot 2/3 = f/w layer-2 inputs. Slab s
    occupies rows (8s mod 32)..+7; convs read a 10-row window with the row
    halo coming from neighbouring slabs' rows already in the ring, so each
    PSUM eviction is ONE engine instruction (no neighbour-row writes).
  - conv = banded matmuls (K=120, M=96) per ky; ky=1 issued first covering
    all 8 out rows (start=True), ky=0/2 accumulate partial row ranges
    (image-edge rows dropped entirely: PSUM has_written handles it).
    Ring-wrap windows split into 2 accumulating matmuls.
  - x-halo strips: SBUF->SBUF DMAs using a flat-shifted AP (dst flat+1 =
    src flat) covering both f and w slots in one DMA; the row-boundary
    smear lands exactly on the zero-pad columns and is re-zeroed by tiny
    gpsimd memsets.
  - layer-0 im2col r0 (bf16, host-built) loaded 4 slabs per DMA; outputs
    staged in 64-row rings and shipped 32 rows per DMA.
  - softmax: E=max(exp(psw),1) (ACT exp + DVE max); EP=(psf max 0)*E in one
    DVE scalar_tensor_tensor; 1/sum via DVE reciprocal_approx_fast.
  - chroma conv(rb-g)+green_add folded host-side into wchm@r0 + wg@G.
  - DMA issue cost (~0.85us each on the issuing queue) spread: SP gets
    g-strips/rt/strips-A, gpsimd gets strips-B (swdge), ACT gets outputs.
  - host does layer-0 im2col and the final 2x2 pixel-shuffle.
"""

import sys

sys.path.insert(0, "/opt/trn_rl_repo")

import ml_dtypes
import numpy as np

import concourse.bass as bass
import concourse.mybir as mybir
import concourse.tile as tile
from concourse import bacc
from concourse.bass_utils import run_bass_kernel_spmd

N_CORES = 8
B_PC = 2
H = W = 512
NW = 64
NSLAB = 64
CH = 12
RR = 128   # activation ring rows (16 slabs)
RRG = 128  # g ring rows (16 slabs)
GP = 36    # g ring partitions: main [0:16), xa0 strip [32:34), xa9 [34:36)
F32 = mybir.dt.float32
BF16 = mybir.dt.bfloat16
NPBF16 = ml_dtypes.bfloat16


def _rbloc(xa, c):
    if xa == 0:
        return 16 + c
    if xa == 9:
        return 18 + c
    return (xa - 1) * 2 + c


def _rloc(xa, ci):
    if xa == 0:
        return 96 + ci
    if xa == 9:
        return 108 + ci
    return (xa - 1) * 12 + ci


def _r0loc(ky, ci, xa):
    if ky == 0:
        if ci == 0:
            return xa
        if ci == 3:
            return 10 + xa
        return 20 + _rbloc(xa, ci - 1)
    if ky == 1:
        if ci == 0:
            return 40 + xa
        if ci == 3:
            return 50 + xa
        return 64 + _rbloc(xa, ci - 1)
    if ci == 0:
        return 84 + xa
    if ci == 3:
        return 94 + xa
    return 104 + _rbloc(xa, ci - 1)


def build_r0(mosaic):
    B = mosaic.shape[0]
    mp = np.zeros((B, 4, H + 2, W + 2), np.float32)
    mp[:, :, 1 : H + 1, 1 : W + 1] = mosaic
    r0 = np.zeros((B, 128, H, NW), np.float32)
    for ky in range(3):
        for ci in range(4):
            for xa in range(10):
                r0[:, _r0loc(ky, ci, xa)] = mp[:, ci, ky : ky + H, xa : xa + 8 * NW : 8]
    return r0


def build_w_l0(wt):
    W_ = np.zeros((128, 128), np.float32)
    for ky in range(3):
        for ci in range(4):
            for xa in range(10):
                for xo in range(8):
                    kx = xa - xo
                    if 0 <= kx <= 2:
                        for co in range(CH):
                            W_[_r0loc(ky, ci, xa), xo * 12 + co] = wt[co, ci, ky, kx]
    return W_


def build_w_int(wt):
    W_ = np.zeros((3, 120, 128), np.float32)
    for ky in range(3):
        for xa in range(10):
            for xo in range(8):
                kx = xa - xo
                if 0 <= kx <= 2:
                    k = _rloc(xa, 0)
                    W_[ky, k : k + 12, xo * 12 : xo * 12 + 12] = wt[:, :, ky, kx].T
    return W_


def build_w_sums():
    wse = np.zeros((96, 8), np.float32)
    wsep = np.zeros((96, 16), np.float32)
    wbc = np.zeros((8, 16), np.float32)
    for xo in range(8):
        for co in range(CH):
            wse[xo * 12 + co, xo] = 1.0
            wsep[xo * 12 + co, xo * 2 + (co >= 6)] = 1.0
        wbc[xo, xo * 2 : xo * 2 + 2] = 1.0
    return wse, wsep, wbc


def _gloc(xa, c):
    if xa == 0:
        return 32 + c
    if xa == 9:
        return 34 + c
    return (xa - 1) * 2 + c


def build_w_chroma(cw0):
    # chroma_pred = conv(rb) - conv(g) + green_add, folded into:
    #   wg[ky] @ G rows   (the -conv(g) part + green_add's g1/g0 picks)
    #   wchm  @ r0 rows   (the +conv(rb) part via r0's per-ky rb replicas
    #                      + green_add's m0/m3 picks)
    wg = np.zeros((3, 36, 48), np.float32)
    for ky in range(3):
        for xa in range(10):
            for xo in range(8):
                kx = xa - xo
                if 0 <= kx <= 2:
                    for co in range(6):
                        for d in range(2):
                            wg[ky, _gloc(xa, d), xo * 6 + co] = -cw0[co, d, ky, kx]
    for xo in range(8):
        wg[1, _gloc(xo + 1, 1), xo * 6 + 1] += 1.0
        wg[1, _gloc(xo + 1, 0), xo * 6 + 4] += 1.0
    wchm = np.zeros((128, 48), np.float32)
    for xo in range(8):
        xa = xo + 1
        wchm[_r0loc(1, 0, xa), xo * 6 + 0] = 1.0
        wchm[_r0loc(1, 0, xa), xo * 6 + 3] = 1.0
        wchm[_r0loc(1, 3, xa), xo * 6 + 2] = 1.0
        wchm[_r0loc(1, 3, xa), xo * 6 + 5] = 1.0
    for ky in range(3):
        for xa in range(10):
            for xo in range(8):
                kx = xa - xo
                if 0 <= kx <= 2:
                    for co in range(6):
                        for d in range(2):
                            wchm[_r0loc(ky, d + 1, xa), xo * 6 + co] += cw0[
                                co, d, ky, kx
                            ]
    return wg, wchm


def assemble_output(mosaic, cp_dev, g_dev):
    B = mosaic.shape[0]
    cp_dev = np.asarray(cp_dev, np.float32)
    g_dev = np.asarray(g_dev, np.float32)
    cp = cp_dev.reshape(B, 8, 6, H, NW).transpose(0, 2, 3, 4, 1).reshape(B, 6, H, W)
    g = g_dev.reshape(B, 8, 2, H, NW).transpose(0, 2, 3, 4, 1).reshape(B, 2, H, W)
    m = mosaic
    out = np.empty((B, 3, 2 * H, 2 * W), np.float32)
    out[:, 0, 0::2, 0::2] = cp[:, 0]
    out[:, 0, 0::2, 1::2] = m[:, 1]
    out[:, 0, 1::2, 0::2] = cp[:, 1]
    out[:, 0, 1::2, 1::2] = cp[:, 2]
    out[:, 1, 0::2, 0::2] = m[:, 0]
    out[:, 1, 0::2, 1::2] = g[:, 0]
    out[:, 1, 1::2, 0::2] = g[:, 1]
    out[:, 1, 1::2, 1::2] = m[:, 3]
    out[:, 2, 0::2, 0::2] = cp[:, 3]
    out[:, 2, 0::2, 1::2] = cp[:, 4]
    out[:, 2, 1::2, 0::2] = m[:, 2]
    out[:, 2, 1::2, 1::2] = cp[:, 5]
    return out


# column offsets inside the packed stationary tensor (conv weights padded
# to 128 columns so bf16 fast-weight-load kicks in)
_WOFF = {"wf0": 0, "ww0": 128, "wf1": 256, "wf2": 640, "ww1": 1024,
         "ww2": 1408, "wse": 1792, "wsep": 1800, "wbc": 1816, "wg": 1832,
         "wchm": 1976}
_WCOLS = 2024


def pack_stationaries(st):
    wp = np.zeros((128, _WCOLS), np.float32)
    wp[:, 0:128] = st["wf0"]
    wp[:, 128:256] = st["ww0"]
    for nm in ("wf1", "wf2", "ww1", "ww2"):
        o = _WOFF[nm]
        for ky in range(3):
            wp[0:120, o + 128 * ky : o + 128 * (ky + 1)] = st[nm][ky]
    o = _WOFF["wse"]; wp[0:96, o : o + 8] = st["wse"]
    o = _WOFF["wsep"]; wp[0:96, o : o + 16] = st["wsep"]
    o = _WOFF["wbc"]; wp[0:8, o : o + 16] = st["wbc"]
    o = _WOFF["wg"]
    for ky in range(3):
        wp[0:36, o + 48 * ky : o + 48 * (ky + 1)] = st["wg"][ky]
    o = _WOFF["wchm"]; wp[:, o : o + 48] = st["wchm"]
    return wp


def _row_pieces(base, n, ring):
    """Split ring-row window [base, base+n) (mod ring) into linear pieces."""
    base %= ring
    if base + n <= ring:
        return [(base, n)]
    return [(base, ring - base), (0, n - (ring - base))]


def build_program():
    from contextlib import ExitStack

    nc = bacc.Bacc(
        "TRN2", target_bir_lowering=False, debug=False, num_devices=N_CORES
    )
    r0 = nc.declare_dram_parameter("r0", [B_PC, 128, H, NW], BF16, isOutput=False)
    wpack = nc.declare_dram_parameter("wpack", [128, _WCOLS], BF16, isOutput=False)
    out_cp = nc.declare_dram_parameter("out_cp", [B_PC, 48, H, NW], BF16, isOutput=True)
    out_g = nc.declare_dram_parameter("out_g", [B_PC, 16, H, NW], BF16, isOutput=True)

    Relu = mybir.ActivationFunctionType.Relu
    Exp = mybir.ActivationFunctionType.Exp
    Copy = mybir.ActivationFunctionType.Copy
    MAX = mybir.AluOpType.max
    MULT = mybir.AluOpType.mult
    NSTEPS = B_PC * NSLAB

    with tile.TileContext(nc) as tc, ExitStack() as ctx:
        const = ctx.enter_context(tc.tile_pool(name="const", bufs=1))
        r0pool = ctx.enter_context(tc.tile_pool(name="r0pool", bufs=4))
        p_act = ctx.enter_context(tc.tile_pool(name="acts", bufs=3))
        ps_mm = ctx.enter_context(tc.tile_pool(name="psmm", bufs=4, space="PSUM"))
        ps_sm = ctx.enter_context(tc.tile_pool(name="pssm", bufs=2, space="PSUM"))
        ps_cp = ctx.enter_context(tc.tile_pool(name="pscp", bufs=2, space="PSUM"))

        WC = const.tile([128, _WCOLS], BF16, tag="wpack_sb", name="wpack_sb")
        nc.sync.dma_start(out=WC[:], in_=wpack[:])
        RNG = const.tile([120, 4, RR, NW], BF16, tag="ring", name="ring")
        G = const.tile([GP, RRG, NW], BF16, tag="gring", name="gring")
        STG = const.tile([48, RRG, NW], BF16, tag="stg", name="stg")

        nc.vector.memset(G[0:GP, :, :], 0.0)
        # one-time zero of the x-pad columns inside the RNG halo strips
        # (engine memsets can't start at partition 108; DMA can)
        ZZ = const.tile([16, 4 * RR], BF16, tag="zz", name="zz")
        nc.vector.memset(ZZ[:], 0.0)
        nc.sync.dma_start(out=RNG[96:108, :, :, 0:1], in_=ZZ[0:12, :])
        nc.sync.dma_start(out=RNG[108:120, :, :, 63:64], in_=ZZ[0:12, :])

        sb = {
            "wf0": WC[:, 0:128],
            "ww0": WC[:, 128:256],
            "wse": WC[0:96, _WOFF["wse"] : _WOFF["wse"] + 8],
            "wsep": WC[0:96, _WOFF["wsep"] : _WOFF["wsep"] + 16],
            "wbc": WC[0:8, _WOFF["wbc"] : _WOFF["wbc"] + 16],
            "wchm": WC[:, _WOFF["wchm"] : _WOFF["wchm"] + 48],
        }

        def wky(nm, ky):
            o = _WOFF[nm]
            if nm == "wg":
                return WC[0:36, o + 48 * ky : o + 48 * (ky + 1)]
            return WC[0:120, o + 128 * ky : o + 128 * (ky + 1)]

        def conv_pieces(s, ring):
            """(ky, in_row, out_row, nrows) pieces; ky=1 first (always full).

            out row r sums input rows 8s+r+ky-1; image-edge taps (row -1 /
            row H) are dropped entirely (PSUM has_written handles partial
            accumulation); ring-wrap windows split into two pieces."""
            sl = s % NSLAB
            out = [(1, (8 * s) % ring, 0, 8)]
            if sl == 0:
                out.append((0, (8 * s) % ring, 1, 7))
            else:
                ro = 0
                for rb, n in _row_pieces(8 * s - 1, 8, ring):
                    out.append((0, rb, ro, n))
                    ro += n
            if sl == NSLAB - 1:
                out.append((2, (8 * s + 1) % ring, 0, 7))
            else:
                ro = 0
                for rb, n in _row_pieces(8 * s + 1, 8, ring):
                    out.append((2, rb, ro, n))
                    ro += n
            return out

        def conv_ring(nm, slot, s):
            ps = ps_mm.tile([128, 8, NW], F32, tag="mm96", name="psc")
            pieces = conv_pieces(s, RR)
            for i, (ky, ri, ro, n) in enumerate(pieces):
                nc.tensor.matmul(
                    ps[:, ro : ro + n, :],
                    wky(nm, ky),
                    RNG[0:120, slot, ri : ri + n, :],
                    start=(i == 0),
                    stop=(i == len(pieces) - 1),
                )
            return ps

        def evict(ps, slot, s, eng):
            rows = (8 * s) % RR
            out = RNG[0:96, slot, rows : rows + 8, :]
            if eng == "act":
                nc.scalar.activation(out=out, in_=ps[0:96], func=Relu)
            else:
                nc.vector.tensor_relu(out=out, in_=ps[0:96])

        def ring_strips(b, nrows, slots, eng):
            """x-halo strips for ring-row window [b, b+nrows) per slot.
            Plain 2D shifted copies: pad cols (xa0 col 0 / xa9 col 63) are
            never touched, so the one-time init zeros persist."""
            for slot in slots:
                for rb, n in _row_pieces(b, nrows, RR):
                    eng.dma_start(
                        out=RNG[96:108, slot, rb : rb + n, 1:NW],
                        in_=RNG[84:96, slot, rb : rb + n, 0 : NW - 1],
                    )
                    eng.dma_start(
                        out=RNG[108:120, slot, rb : rb + n, 0 : NW - 1],
                        in_=RNG[0:12, slot, rb : rb + n, 1:NW],
                    )

        def g_strips(b, nrows, eng):
            for rb, n in _row_pieces(b, nrows, RRG):
                eng.dma_start(
                    out=G[32:34, rb : rb + n, 1:NW],
                    in_=G[14:16, rb : rb + n, 0 : NW - 1],
                )
                eng.dma_start(
                    out=G[34:36, rb : rb + n, 0 : NW - 1],
                    in_=G[0:2, rb : rb + n, 1:NW],
                )

        rt4s = {}
        es, eps, rcps, smas = {}, {}, {}, {}

        for T in range(NSTEPS + 19):
            # g strips for chroma readers at T..T+3 (slabs T-16..T-13);
            # content is >=2 iterations old -> issues immediately
            if T % 4 == 0 and T - 14 >= 0 and T - 17 < NSTEPS:
                g_strips((8 * max(T - 17, 0)) % RRG, 33, nc.gpsimd)

            # stage sD = T-9: softmax sums for slab sD (E/EP made last iter)
            sD = T - 11
            if 0 <= sD < NSTEPS:
                sm = ps_sm.tile([128, 8, NW], F32, tag="smA", name="smA")
                smas[sD] = sm
                nc.tensor.matmul(sm[0:8], sb["wse"], es[sD][:], start=True, stop=True)
                nc.tensor.matmul(
                    sm[32:48], sb["wsep"], eps[sD][:], start=True, stop=True
                )
                rcpf = p_act.tile([8, 8, NW], F32, tag="rcpf", name="rcpf")
                nc.vector.reciprocal_approx_fast(out=rcpf[:], in_=sm[0:8])
                rcp = p_act.tile([8, 8, NW], BF16, tag="rcp", name="rcp")
                nc.vector.tensor_copy(out=rcp[:], in_=rcpf[:])
                rcps[sD] = rcp

            # stage sE = T-10: softmax broadcast + g rows for slab sE
            sE = T - 12
            if 0 <= sE < NSTEPS:
                sm = smas[sE]
                nc.tensor.matmul(
                    sm[64:80], sb["wbc"], rcps[sE][:], start=True, stop=True
                )
                bcs = p_act.tile([16, 8, NW], BF16, tag="bcs", name="bcs")
                nc.scalar.activation(out=bcs[:], in_=sm[64:80], func=Copy)
                gr = (8 * sE) % RRG
                nc.vector.tensor_mul(G[0:16, gr : gr + 8, :], sm[32:48], bcs[:])

            # stage sA = T: layer-0 convs from r0 (8-slab granule loads)
            s0 = T
            if 0 <= s0 < NSTEPS:
                if s0 % 8 == 0:
                    img = s0 // NSLAB
                    y0 = (s0 % NSLAB) * 8
                    rt4 = r0pool.tile([128, 64, NW], BF16, name="rt4")
                    rt4s[s0 // 8] = rt4
                    nc.sync.dma_start(out=rt4[:], in_=r0[img, :, y0 : y0 + 64, :])
                rt4 = rt4s[s0 // 8]
                k = s0 % 8
                psf = ps_mm.tile([128, 8, NW], F32, tag="mm96", name="psf0")
                nc.tensor.matmul(
                    psf[:], sb["wf0"], rt4[:, 8 * k : 8 * k + 8, :],
                    start=True, stop=True,
                )
                evict(psf, 0, s0, "act")
                psw = ps_mm.tile([128, 8, NW], F32, tag="mm96", name="psw0")
                nc.tensor.matmul(
                    psw[:], sb["ww0"], rt4[:, 8 * k : 8 * k + 8, :],
                    start=True, stop=True,
                )
                evict(psw, 1, s0, "dve")

            # strips for L1 readers at T..T+3 (slabs T-4..T-1)
            if T % 4 == 0 and 0 <= T - 4 < NSTEPS:
                ring_strips((8 * (T - 4)) % RR, 33, (0, 1), nc.sync)

            # stage sB = T-4: layer-1 convs
            s2 = T - 5
            if 0 <= s2 < NSTEPS:
                evict(conv_ring("wf1", 0, s2), 2, s2, "act")
                evict(conv_ring("ww1", 1, s2), 3, s2, "dve")

            # strips for L2 readers at T..T+3 (slabs T-8..T-5)
            if T % 4 == 0 and T - 6 >= 0 and T - 9 < NSTEPS:
                ring_strips((8 * max(T - 9, 0)) % RR, 33, (2, 3), nc.gpsimd)

            # stage sC = T-8: layer-2 convs + E/EP (consumed next iteration)
            s4 = T - 10
            if 0 <= s4 < NSTEPS:
                psf = conv_ring("wf2", 2, s4)
                psw = conv_ring("ww2", 3, s4)
                E0 = p_act.tile([96, 8, NW], BF16, tag="E0", name="E0")
                nc.scalar.activation(out=E0[:], in_=psw[0:96], func=Exp)
                E = p_act.tile([96, 8, NW], BF16, tag="E", name="E")
                nc.vector.tensor_scalar_max(E[:], E0[:], 1.0)
                es[s4] = E
                EP = p_act.tile([96, 8, NW], BF16, tag="EP", name="EP")
                nc.vector.scalar_tensor_tensor(EP[:], psf[0:96], 0.0, E[:], MAX, MULT)
                eps[s4] = EP

            # stage sF = T-16: chroma + staging + batched outputs
            s6 = T - 18
            if 0 <= s6 < NSTEPS:
                pc = ps_cp.tile([48, 8, NW], F32, tag="cp", name="pc")
                rt4 = rt4s[s6 // 8]
                k = s6 % 8
                nc.tensor.matmul(
                    pc[:], sb["wchm"], rt4[:, 8 * k : 8 * k + 8, :],
                    start=True, stop=False,
                )
                pieces = conv_pieces(s6, RRG)
                for i, (ky, ri, ro, n) in enumerate(pieces):
                    nc.tensor.matmul(
                        pc[:, ro : ro + n, :],
                        wky("wg", ky),
                        G[0:36, ri : ri + n, :],
                        start=False,
                        stop=(i == len(pieces) - 1),
                    )
                sg = (8 * s6) % RRG
                nc.scalar.activation(out=STG[:, sg : sg + 8, :], in_=pc[:], func=Copy)
                if s6 % 8 == 7:
                    img = s6 // NSLAB
                    y0 = ((s6 - 7) % NSLAB) * 8
                    blk = (8 * (s6 - 7)) % RRG
                    nc.scalar.dma_start(
                        out=out_cp[img, :, y0 : y0 + 64, :],
                        in_=STG[0:48, blk : blk + 64, :],
                    )
                    nc.scalar.dma_start(
                        out=out_g[img, :, y0 : y0 + 64, :],
                        in_=G[0:16, blk : blk + 64, :],
                    )
                rt4s.pop(s6 // 8 - 3, None)
                for dd in (es, eps, rcps, smas):
                    dd.pop(s6, None)

    nc.compile()
    return nc


_CACHE = {}


def kernel(mosaic, fw0, fw1, fw2, ww0, ww1, ww2, cw0, _trace=False):
    mosaic = np.asarray(mosaic, np.float32)
    r0_all = build_r0(mosaic).astype(NPBF16)

    stat = {
        "wf0": build_w_l0(np.asarray(fw0, np.float32)),
        "ww0": build_w_l0(np.asarray(ww0, np.float32)),
        "wf1": build_w_int(np.asarray(fw1, np.float32)),
        "wf2": build_w_int(np.asarray(fw2, np.float32)),
        "ww1": build_w_int(np.asarray(ww1, np.float32)),
        "ww2": build_w_int(np.asarray(ww2, np.float32)),
    }
    stat["wse"], stat["wsep"], stat["wbc"] = build_w_sums()
    stat["wg"], stat["wchm"] = build_w_chroma(np.asarray(cw0, np.float32))
    wpack = pack_stationaries(stat).astype(NPBF16)

    if "nc" not in _CACHE:
        _CACHE["nc"] = build_program()
    nc = _CACHE["nc"]

    in_maps = []
    for c in range(N_CORES):
        in_maps.append(
            {"r0": np.ascontiguousarray(r0_all[c * B_PC : (c + 1) * B_PC]),
             "wpack": wpack}
        )

    res = run_bass_kernel_spmd(nc, in_maps, list(range(N_CORES)), trace=_trace)
    outs = []
    for c in range(N_CORES):
        outs.append(
            assemble_output(
                mosaic[c * B_PC : (c + 1) * B_PC],
                res.results[c]["out_cp"],
                res.results[c]["out_g"],
            )
        )
    full = np.concatenate(outs, axis=0)
    if _trace:
        return full, res
    return full


# revision 18
# speedup vs baseline: 4.0379x; 1.4504x over previous
"""Trainium2 Bass kernel for BasicQuadRGBModel (quad-Bayer demosaic CNN).

v5: bf16 + ring buffers + batched/spread DMA issue.
  - activations live in ONE ring tensor RNG [120p, 4 slots, 32 rows, 64]:
    slot 0/1 = f/w layer-1 inputs, slot 2/3 = f/w layer-2 inputs. Slab s
    occupies rows (8s mod 32)..+7; convs read a 10-row window with the row
    halo coming from neighbouring slabs' rows already in the ring, so each
    PSUM eviction is ONE engine instruction (no neighbour-row writes).
  - conv = banded matmuls (K=120, M=96) per ky; ky=1 issued first covering
    all 8 out rows (start=True), ky=0/2 accumulate partial row ranges
    (image-edge rows dropped entirely: PSUM has_written handles it).
    Ring-wrap windows split into 2 accumulating matmuls.
  - x-halo strips: SBUF->SBUF DMAs using a flat-shifted AP (dst flat+1 =
    src flat) covering both f and w slots in one DMA; the row-boundary
    smear lands exactly on the zero-pad columns and is re-zeroed by tiny
    gpsimd memsets.
  - layer-0 im2col r0 (bf16, host-built) loaded 4 slabs per DMA; outputs
    staged in 64-row rings and shipped 32 rows per DMA.
  - softmax: E=max(exp(psw),1) (ACT exp + DVE max); EP=(psf max 0)*E in one
    DVE scalar_tensor_tensor; 1/sum via DVE reciprocal_approx_fast.
  - chroma conv(rb-g)+green_add folded host-side into wchm@r0 + wg@G.
  - DMA issue cost (~0.85us each on the issuing queue) spread: SP gets
    g-strips/rt/strips-A, gpsimd gets strips-B (swdge), ACT gets outputs.
  - host does layer-0 im2col and the final 2x2 pixel-shuffle.
"""

import sys

sys.path.insert(0, "/opt/trn_rl_repo")

import ml_dtypes
import numpy as np

import concourse.bass as bass
import concourse.mybir as mybir
import concourse.tile as tile
from concourse import bacc
from concourse.bass_utils import run_bass_kernel_spmd

N_CORES = 8
B_PC = 2
H = W = 512
NW = 64
NSLAB = 64
CH = 12
RR = 128   # activation ring rows (16 slabs)
RRG = 128  # g ring rows (16 slabs)
GP = 36    # g ring partitions: main [0:16), xa0 strip [32:34), xa9 [34:36)
F32 = mybir.dt.float32
BF16 = mybir.dt.bfloat16
NPBF16 = ml_dtypes.bfloat16


def _rbloc(xa, c):
    if xa == 0:
        return 16 + c
    if xa == 9:
        return 18 + c
    return (xa - 1) * 2 + c


def _rloc(xa, ci):
    if xa == 0:
        return 96 + ci
    if xa == 9:
        return 108 + ci
    return (xa - 1) * 12 + ci


def _r0loc(ky, ci, xa):
    if ky == 0:
        if ci == 0:
            return xa
        if ci == 3:
            return 10 + xa
        return 20 + _rbloc(xa, ci - 1)
    if ky == 1:
        if ci == 0:
            return 40 + xa
        if ci == 3:
            return 50 + xa
        return 64 + _rbloc(xa, ci - 1)
    if ci == 0:
        return 84 + xa
    if ci == 3:
        return 94 + xa
    return 104 + _rbloc(xa, ci - 1)


def build_r0(mosaic):
    B = mosaic.shape[0]
    mp = np.zeros((B, 4, H + 2, W + 2), np.float32)
    mp[:, :, 1 : H + 1, 1 : W + 1] = mosaic
    r0 = np.zeros((B, 128, H, NW), np.float32)
    for ky in range(3):
        for ci in range(4):
            for xa in range(10):
                r0[:, _r0loc(ky, ci, xa)] = mp[:, ci, ky : ky + H, xa : xa + 8 * NW : 8]
    return r0


def build_w_l0(wt):
    W_ = np.zeros((128, 128), np.float32)
    for ky in range(3):
        for ci in range(4):
            for xa in range(10):
                for xo in range(8):
                    kx = xa - xo
                    if 0 <= kx <= 2:
                        for co in range(CH):
                            W_[_r0loc(ky, ci, xa), xo * 12 + co] = wt[co, ci, ky, kx]
    return W_


def build_w_int(wt):
    W_ = np.zeros((3, 120, 128), np.float32)
    for ky in range(3):
        for xa in range(10):
            for xo in range(8):
                kx = xa - xo
                if 0 <= kx <= 2:
                    k = _rloc(xa, 0)
                    W_[ky, k : k + 12, xo * 12 : xo * 12 + 12] = wt[:, :, ky, kx].T
    return W_


def build_w_sums():
    wse = np.zeros((96, 8), np.float32)
    wsep = np.zeros((96, 16), np.float32)
    wbc = np.zeros((8, 16), np.float32)
    for xo in range(8):
        for co in range(CH):
            wse[xo * 12 + co, xo] = 1.0
            wsep[xo * 12 + co, xo * 2 + (co >= 6)] = 1.0
        wbc[xo, xo * 2 : xo * 2 + 2] = 1.0
    return wse, wsep, wbc


def _gloc(xa, c):
    if xa == 0:
        return 32 + c
    if xa == 9:
        return 34 + c
    return (xa - 1) * 2 + c


def build_w_chroma(cw0):
    # chroma_pred = conv(rb) - conv(g) + green_add, folded into:
    #   wg[ky] @ G rows   (the -conv(g) part + green_add's g1/g0 picks)
    #   wchm  @ r0 rows   (the +conv(rb) part via r0's per-ky rb replicas
    #                      + green_add's m0/m3 picks)
    wg = np.zeros((3, 36, 48), np.float32)
    for ky in range(3):
        for xa in range(10):
            for xo in range(8):
                kx = xa - xo
                if 0 <= kx <= 2:
                    for co in range(6):
                        for d in range(2):
                            wg[ky, _gloc(xa, d), xo * 6 + co] = -cw0[co, d, ky, kx]
    for xo in range(8):
        wg[1, _gloc(xo + 1, 1), xo * 6 + 1] += 1.0
        wg[1, _gloc(xo + 1, 0), xo * 6 + 4] += 1.0
    wchm = np.zeros((128, 48), np.float32)
    for xo in range(8):
        xa = xo + 1
        wchm[_r0loc(1, 0, xa), xo * 6 + 0] = 1.0
        wchm[_r0loc(1, 0, xa), xo * 6 + 3] = 1.0
        wchm[_r0loc(1, 3, xa), xo * 6 + 2] = 1.0
        wchm[_r0loc(1, 3, xa), xo * 6 + 5] = 1.0
    for ky in range(3):
        for xa in range(10):
            for xo in range(8):
                kx = xa - xo
                if 0 <= kx <= 2:
                    for co in range(6):
                        for d in range(2):
                            wchm[_r0loc(ky, d + 1, xa), xo * 6 + co] += cw0[
                                co, d, ky, kx
                            ]
    return wg, wchm


def assemble_output(mosaic, cp_dev, g_dev):
    B = mosaic.shape[0]
    cp_dev = np.asarray(cp_dev, np.float32)
    g_dev = np.asarray(g_dev, np.float32)
    cp = cp_dev.reshape(B, 8, 6, H, NW).transpose(0, 2, 3, 4, 1).reshape(B, 6, H, W)
    g = g_dev.reshape(B, 8, 2, H, NW).transpose(0, 2, 3, 4, 1).reshape(B, 2, H, W)
    m = mosaic
    out = np.empty((B, 3, 2 * H, 2 * W), np.float32)
    out[:, 0, 0::2, 0::2] = cp[:, 0]
    out[:, 0, 0::2, 1::2] = m[:, 1]
    out[:, 0, 1::2, 0::2] = cp[:, 1]
    out[:, 0, 1::2, 1::2] = cp[:, 2]
    out[:, 1, 0::2, 0::2] = m[:, 0]
    out[:, 1, 0::2, 1::2] = g[:, 0]
    out[:, 1, 1::2, 0::2] = g[:, 1]
    out[:, 1, 1::2, 1::2] = m[:, 3]
    out[:, 2, 0::2, 0::2] = cp[:, 3]
    out[:, 2, 0::2, 1::2] = cp[:, 4]
    out[:, 2, 1::2, 0::2] = m[:, 2]
    out[:, 2, 1::2, 1::2] = cp[:, 5]
    return out


# column offsets inside the packed stationary tensor (conv weights padded
# to 128 columns so bf16 fast-weight-load kicks in)
_WOFF = {"wf0": 0, "ww0": 128, "wf1": 256, "wf2": 640, "ww1": 1024,
         "ww2": 1408, "wse": 1792, "wsep": 1800, "wbc": 1816, "wg": 1832,
         "wchm": 1976}
_WCOLS = 2024


def pack_stationaries(st):
    wp = np.zeros((128, _WCOLS), np.float32)
    wp[:, 0:128] = st["wf0"]
    wp[:, 128:256] = st["ww0"]
    for nm in ("wf1", "wf2", "ww1", "ww2"):
        o = _WOFF[nm]
        for ky in range(3):
            wp[0:120, o + 128 * ky : o + 128 * (ky + 1)] = st[nm][ky]
    o = _WOFF["wse"]; wp[0:96, o : o + 8] = st["wse"]
    o = _WOFF["wsep"]; wp[0:96, o : o + 16] = st["wsep"]
    o = _WOFF["wbc"]; wp[0:8, o : o + 16] = st["wbc"]
    o = _WOFF["wg"]
    for ky in range(3):
        wp[0:36, o + 48 * ky : o + 48 * (ky + 1)] = st["wg"][ky]
    o = _WOFF["wchm"]; wp[:, o : o + 48] = st["wchm"]
    return wp


def _row_pieces(base, n, ring):
    """Split ring-row window [base, base+n) (mod ring) into linear pieces."""
    base %= ring
    if base + n <= ring:
        return [(base, n)]
    return [(base, ring - base), (0, n - (ring - base))]


def build_program():
    from contextlib import ExitStack

    nc = bacc.Bacc(
        "TRN2", target_bir_lowering=False, debug=False, num_devices=N_CORES
    )
    r0 = nc.declare_dram_parameter("r0", [B_PC, 128, H, NW], BF16, isOutput=False)
    wpack = nc.declare_dram_parameter("wpack", [128, _WCOLS], BF16, isOutput=False)
    out_cp = nc.declare_dram_parameter("out_cp", [B_PC, 48, H, NW], BF16, isOutput=True)
    out_g = nc.declare_dram_parameter("out_g", [B_PC, 16, H, NW], BF16, isOutput=True)

    Relu = mybir.ActivationFunctionType.Relu
    Exp = mybir.ActivationFunctionType.Exp
    Copy = mybir.ActivationFunctionType.Copy
    MAX = mybir.AluOpType.max
    MULT = mybir.AluOpType.mult
    NSTEPS = B_PC * NSLAB

    with tile.TileContext(nc) as tc, ExitStack() as ctx:
        const = ctx.enter_context(tc.tile_pool(name="const", bufs=1))
        r0pool = ctx.enter_context(tc.tile_pool(name="r0pool", bufs=5))
        p_act = ctx.enter_context(tc.tile_pool(name="acts", bufs=3))
        ps_mm = ctx.enter_context(tc.tile_pool(name="psmm", bufs=4, space="PSUM"))
        ps_sm = ctx.enter_context(tc.tile_pool(name="pssm", bufs=2, space="PSUM"))
        ps_cp = ctx.enter_context(tc.tile_pool(name="pscp", bufs=2, space="PSUM"))

        WC = const.tile([128, _WCOLS], BF16, tag="wpack_sb", name="wpack_sb")
        nc.sync.dma_start(out=WC[:], in_=wpack[:])
        RNG = const.tile([120, 4, RR, NW], BF16, tag="ring", name="ring")
        G = const.tile([GP, RRG, NW], BF16, tag="gring", name="gring")
        STG = const.tile([48, RRG, NW], BF16, tag="stg", name="stg")

        nc.vector.memset(G[0:GP, :, :], 0.0)
        # one-time zero of the x-pad columns inside the RNG halo strips
        # (engine memsets can't start at partition 108; DMA can)
        ZZ = const.tile([16, 4 * RR], BF16, tag="zz", name="zz")
        nc.vector.memset(ZZ[:], 0.0)
        nc.sync.dma_start(out=RNG[96:108, :, :, 0:1], in_=ZZ[0:12, :])
        nc.sync.dma_start(out=RNG[108:120, :, :, 63:64], in_=ZZ[0:12, :])

        sb = {
            "wf0": WC[:, 0:128],
            "ww0": WC[:, 128:256],
            "wse": WC[0:96, _WOFF["wse"] : _WOFF["wse"] + 8],
            "wsep": WC[0:96, _WOFF["wsep"] : _WOFF["wsep"] + 16],
            "wbc": WC[0:8, _WOFF["wbc"] : _WOFF["wbc"] + 16],
            "wchm": WC[:, _WOFF["wchm"] : _WOFF["wchm"] + 48],
        }

        def wky(nm, ky):
            o = _WOFF[nm]
            if nm == "wg":
                return WC[0:36, o + 48 * ky : o + 48 * (ky + 1)]
            return WC[0:120, o + 128 * ky : o + 128 * (ky + 1)]

        def conv_pieces(s, ring):
            """(ky, in_row, out_row, nrows) pieces; ky=1 first (always full).

            out row r sums input rows 8s+r+ky-1; image-edge taps (row -1 /
            row H) are dropped entirely (PSUM has_written handles partial
            accumulation); ring-wrap windows split into two pieces."""
            sl = s % NSLAB
            out = [(1, (8 * s) % ring, 0, 8)]
            if sl == 0:
                out.append((0, (8 * s) % ring, 1, 7))
            else:
                ro = 0
                for rb, n in _row_pieces(8 * s - 1, 8, ring):
                    out.append((0, rb, ro, n))
                    ro += n
            if sl == NSLAB - 1:
                out.append((2, (8 * s + 1) % ring, 0, 7))
            else:
                ro = 0
                for rb, n in _row_pieces(8 * s + 1, 8, ring):
                    out.append((2, rb, ro, n))
                    ro += n
            return out

        def conv_ring(nm, slot, s):
            ps = ps_mm.tile([128, 8, NW], F32, tag="mm96", name="psc")
            pieces = conv_pieces(s, RR)
            for i, (ky, ri, ro, n) in enumerate(pieces):
                nc.tensor.matmul(
                    ps[:, ro : ro + n, :],
                    wky(nm, ky),
                    RNG[0:120, slot, ri : ri + n, :],
                    start=(i == 0),
                    stop=(i == len(pieces) - 1),
                )
            return ps

        def evict(ps, slot, s, eng):
            rows = (8 * s) % RR
            out = RNG[0:96, slot, rows : rows + 8, :]
            if eng == "act":
                nc.scalar.activation(out=out, in_=ps[0:96], func=Relu)
            else:
                nc.vector.tensor_relu(out=out, in_=ps[0:96])

        def ring_strips(b, nrows, slots, eng):
            """x-halo strips for ring-row window [b, b+nrows) per slot.
            Plain 2D shifted copies: pad cols (xa0 col 0 / xa9 col 63) are
            never touched, so the one-time init zeros persist."""
            for slot in slots:
                for rb, n in _row_pieces(b, nrows, RR):
                    eng.dma_start(
                        out=RNG[96:108, slot, rb : rb + n, 1:NW],
                        in_=RNG[84:96, slot, rb : rb + n, 0 : NW - 1],
                    )
                    eng.dma_start(
                        out=RNG[108:120, slot, rb : rb + n, 0 : NW - 1],
                        in_=RNG[0:12, slot, rb : rb + n, 1:NW],
                    )

        def g_strips(b, nrows, eng):
            for rb, n in _row_pieces(b, nrows, RRG):
                eng.dma_start(
                    out=G[32:34, rb : rb + n, 1:NW],
                    in_=G[14:16, rb : rb + n, 0 : NW - 1],
                )
                eng.dma_start(
                    out=G[34:36, rb : rb + n, 0 : NW - 1],
                    in_=G[0:2, rb : rb + n, 1:NW],
                )

        rt4s = {}
        es, eps, rcps, smas = {}, {}, {}, {}

        for T in range(NSTEPS + 28):
            # g strips for chroma readers at T..T+3 (slabs T-16..T-13);
            # content is >=2 iterations old -> issues immediately
            if T % 4 == 0 and T - 22 >= 0 and T - 25 < NSTEPS:
                g_strips((8 * max(T - 25, 0)) % RRG, 33, nc.gpsimd)

            # stage sD = T-9: softmax sums for slab sD (E/EP made last iter)
            sD = T - 19
            if 0 <= sD < NSTEPS:
                sm = ps_sm.tile([128, 8, NW], F32, tag="smA", name="smA")
                smas[sD] = sm
                nc.tensor.matmul(sm[0:8], sb["wse"], es[sD][:], start=True, stop=True)
                nc.tensor.matmul(
                    sm[32:48], sb["wsep"], eps[sD][:], start=True, stop=True
                )
                rcpf = p_act.tile([8, 8, NW], F32, tag="rcpf", name="rcpf")
                nc.vector.reciprocal_approx_fast(out=rcpf[:], in_=sm[0:8])
                rcp = p_act.tile([8, 8, NW], BF16, tag="rcp", name="rcp")
                nc.vector.tensor_copy(out=rcp[:], in_=rcpf[:])
                rcps[sD] = rcp

            # stage sE = T-10: softmax broadcast + g rows for slab sE
            sE = T - 20
            if 0 <= sE < NSTEPS:
                sm = smas[sE]
                nc.tensor.matmul(
                    sm[64:80], sb["wbc"], rcps[sE][:], start=True, stop=True
                )
                bcs = p_act.tile([16, 8, NW], BF16, tag="bcs", name="bcs")
                nc.scalar.activation(out=bcs[:], in_=sm[64:80], func=Copy)
                gr = (8 * sE) % RRG
                nc.vector.tensor_mul(G[0:16, gr : gr + 8, :], sm[32:48], bcs[:])

            # stage sA = T: layer-0 convs from r0 (8-slab granule loads)
            s0 = T
            if 0 <= s0 < NSTEPS:
                if s0 % 8 == 0:
                    img = s0 // NSLAB
                    y0 = (s0 % NSLAB) * 8
                    rt4 = r0pool.tile([128, 64, NW], BF16, name="rt4")
                    rt4s[s0 // 8] = rt4
                    nc.sync.dma_start(out=rt4[:], in_=r0[img, :, y0 : y0 + 64, :])
                rt4 = rt4s[s0 // 8]
                k = s0 % 8
                psf = ps_mm.tile([128, 8, NW], F32, tag="mm96", name="psf0")
                nc.tensor.matmul(
                    psf[:], sb["wf0"], rt4[:, 8 * k : 8 * k + 8, :],
                    start=True, stop=True,
                )
                evict(psf, 0, s0, "act")
                psw = ps_mm.tile([128, 8, NW], F32, tag="mm96", name="psw0")
                nc.tensor.matmul(
                    psw[:], sb["ww0"], rt4[:, 8 * k : 8 * k + 8, :],
                    start=True, stop=True,
                )
                evict(psw, 1, s0, "dve")

            # strips for L1 readers at T..T+3 (slabs T-4..T-1)
            if T % 4 == 0 and T - 5 >= 0 and T - 8 < NSTEPS:
                ring_strips((8 * max(T - 8, 0)) % RR, 33, (0, 1), nc.sync)

            # stage sB = T-4: layer-1 convs
            s2 = T - 9
            if 0 <= s2 < NSTEPS:
                evict(conv_ring("wf1", 0, s2), 2, s2, "act")
                evict(conv_ring("ww1", 1, s2), 3, s2, "dve")

            # strips for L2 readers at T..T+3 (slabs T-8..T-5)
            if T % 4 == 0 and T - 14 >= 0 and T - 17 < NSTEPS:
                ring_strips((8 * max(T - 17, 0)) % RR, 33, (2, 3), nc.gpsimd)

            # stage sC = T-8: layer-2 convs + E/EP (consumed next iteration)
            s4 = T - 18
            if 0 <= s4 < NSTEPS:
                psf = conv_ring("wf2", 2, s4)
                psw = conv_ring("ww2", 3, s4)
                E0 = p_act.tile([96, 8, NW], BF16, tag="E0", name="E0")
                nc.scalar.activation(out=E0[:], in_=psw[0:96], func=Exp)
                E = p_act.tile([96, 8, NW], BF16, tag="E", name="E")
                nc.vector.tensor_scalar_max(E[:], E0[:], 1.0)
                es[s4] = E
                EP = p_act.tile([96, 8, NW], BF16, tag="EP", name="EP")
                nc.vector.scalar_tensor_tensor(EP[:], psf[0:96], 0.0, E[:], MAX, MULT)
                eps[s4] = EP

            # stage sF = T-16: chroma + staging + batched outputs
            s6 = T - 27
            if 0 <= s6 < NSTEPS:
                pc = ps_cp.tile([48, 8, NW], F32, tag="cp", name="pc")
                rt4 = rt4s[s6 // 8]
                k = s6 % 8
                nc.tensor.matmul(
                    pc[:], sb["wchm"], rt4[:, 8 * k : 8 * k + 8, :],
                    start=True, stop=False,
                )
                pieces = conv_pieces(s6, RRG)
                for i, (ky, ri, ro, n) in enumerate(pieces):
                    nc.tensor.matmul(
                        pc[:, ro : ro + n, :],
                        wky("wg", ky),
                        G[0:36, ri : ri + n, :],
                        start=False,
                        stop=(i == len(pieces) - 1),
                    )
                sg = (8 * s6) % RRG
                nc.scalar.activation(out=STG[:, sg : sg + 8, :], in_=pc[:], func=Copy)
                if s6 % 8 == 7:
                    img = s6 // NSLAB
                    y0 = ((s6 - 7) % NSLAB) * 8
                    blk = (8 * (s6 - 7)) % RRG
                    nc.scalar.dma_start(
                        out=out_cp[img, :, y0 : y0 + 64, :],
                        in_=STG[0:48, blk : blk + 64, :],
                    )
                    nc.scalar.dma_start(
                        out=out_g[img, :, y0 : y0 + 64, :],
                        in_=G[0:16, blk : blk + 64, :],
                    )
                rt4s.pop(s6 // 8 - 4, None)
                for dd in (es, eps, rcps, smas):
                    dd.pop(s6, None)

    nc.compile()
    return nc


_CACHE = {}


def kernel(mosaic, fw0, fw1, fw2, ww0, ww1, ww2, cw0, _trace=False):
    mosaic = np.asarray(mosaic, np.float32)
    r0_all = build_r0(mosaic).astype(NPBF16)

    stat = {
        "wf0": build_w_l0(np.asarray(fw0, np.float32)),
        "ww0": build_w_l0(np.asarray(ww0, np.float32)),
        "wf1": build_w_int(np.asarray(fw1, np.float32)),
        "wf2": build_w_int(np.asarray(fw2, np.float32)),
        "ww1": build_w_int(np.asarray(ww1, np.float32)),
        "ww2": build_w_int(np.asarray(ww2, np.float32)),
    }
    stat["wse"], stat["wsep"], stat["wbc"] = build_w_sums()
    stat["wg"], stat["wchm"] = build_w_chroma(np.asarray(cw0, np.float32))
    wpack = pack_stationaries(stat).astype(NPBF16)

    if "nc" not in _CACHE:
        _CACHE["nc"] = build_program()
    nc = _CACHE["nc"]

    in_maps = []
    for c in range(N_CORES):
        in_maps.append(
            {"r0": np.ascontiguousarray(r0_all[c * B_PC : (c + 1) * B_PC]),
             "wpack": wpack}
        )

    res = run_bass_kernel_spmd(nc, in_maps, list(range(N_CORES)), trace=_trace)
    outs = []
    for c in range(N_CORES):
        outs.append(
            assemble_output(
                mosaic[c * B_PC : (c + 1) * B_PC],
                res.results[c]["out_cp"],
                res.results[c]["out_g"],
            )
        )
    full = np.concatenate(outs, axis=0)
    if _trace:
        return full, res
    return full
